# revision 22
# baseline (speedup 1.0000x reference)
"""CPC (contrastive predictive coding) forward pass on 8 Trainium2 NeuronCores.

Data-parallel over the batch: each core processes 8 images (392 patches).
Single SPMD launch; the contrastive targets are exchanged with an on-device
AllGather so each core can normalize its logits over all 3136 negatives.

Encoder conv1 (3->256, k8 s8) runs as 8 PSUM-accumulated matmuls over the
kernel-column index kw with K=(channel, kernel-row)=24 on the partitions and
strided free-dim slices of the patch-row tensor B[(c,kh), (patch,oh,x)] —
this keeps every DMA from HBM at 256B-contiguous runs (no im2col gather).
Border masking is (p+1)*rowmask*colmask - 1 folded into conv1's bias.
"""

import sys
from contextlib import ExitStack
from dataclasses import dataclass

import numpy as np

for _p in ("/opt/trn_rl_repo", "/root/.axon_site/_ro/trn_rl_repo"):
    if _p not in sys.path:
        sys.path.append(_p)

import concourse.bass as bass
import concourse.mybir as mybir
import concourse.tile as tile

f32 = mybir.dt.float32
bf16 = mybir.dt.bfloat16
AF = mybir.ActivationFunctionType
AL = mybir.AluOpType
AX = mybir.AxisListType


@dataclass(frozen=True)
class Cfg:
    ncores: int = 8
    bl: int = 8          # images per core
    latent: int = 2048
    nblk: int = 5        # pixelcnn residual blocks
    use_collective: bool = True
    sim_safe: bool = False   # memset pad partitions so CoreSim sees no uninit reads

    @property
    def mt(self):        # latent 128-tiles
        return self.latent // 128

    @property
    def np_(self):       # patches per core
        return self.bl * 49

    @property
    def rows(self):      # contrastive rows per core (steps 2,3,4 kept rows)
        return self.bl * (4 + 3 + 2) * 7

    @property
    def gcols(self):     # global negatives
        return self.ncores * self.np_


def _apv(base, dims, extra_off=0):
    return bass.AP(tensor=base.tensor, offset=base.offset + extra_off, ap=list(dims))


def _split_multiwait(nc):
    """This env's walrus accepts only one sync-wait per instruction; split
    extras onto single-wait NoOps placed just before."""
    for fn in nc.m.functions:
        for blk in fn.blocks:
            new_insts = []
            for inst in blk.instructions:
                si = inst.sync_info
                if si is not None and len(si.on_wait) > 1:
                    waits = list(si.on_wait)
                    for j, w in enumerate(waits[:-1]):
                        new_insts.append(
                            mybir.InstNoOp(
                                name=f"{inst.name}-wsplit{j}",
                                sync_info=mybir.SyncInfo(on_wait=[w], on_update=[]),
                                bass_nofuse=True,
                                engine=inst.engine,
                            )
                        )
                    inst.sync_info = mybir.SyncInfo(
                        on_wait=[waits[-1]], on_update=list(si.on_update)
                    )
                new_insts.append(inst)
            blk.instructions = new_insts


def build_nc(cfg: Cfg):
    BL, MT, NBLK, NP = cfg.bl, cfg.mt, cfg.nblk, cfg.np_
    LAT = cfg.latent
    KC = MT  # 128-chunks of the latent contraction
    ROWS = cfg.rows
    GC = cfg.gcols
    HI = (4, 3, 2)
    OFFS = (0, BL * 28, BL * 49)  # row offsets of the 3 steps in preds_cat

    nc = bass.Bass()
    dp = nc.declare_dram_parameter
    images = dp("images", [BL, 3, 256, 256], f32, isOutput=False)
    rowmx = dp("rowmx", [BL, 24, 392], f32, isOutput=False)   # (b,(c,kh),(pr,pc,oh))
    colmx = dp("colmx", [BL, 24, 3136], f32, isOutput=False)  # (b,(c,kh),(pr,pc,x))
    w1r = dp("w1r", [24, 8, 256], f32, isOutput=False)      # (c*8+kh, kw, co)
    b1p = dp("b1p", [128, 2], f32, isOutput=False)           # b1 - sum(W1)
    w2t = dp("w2t", [256, LAT], f32, isOutput=False)
    b2 = dp("b2", [128, MT], f32, isOutput=False)
    pc1t = dp("pc1t", [NBLK, LAT, 256], f32, isOutput=False)
    pcb1 = dp("pcb1", [128, NBLK, 2], f32, isOutput=False)
    pc2t = dp("pc2t", [NBLK, 3, 256, 256], f32, isOutput=False)
    pcb2 = dp("pcb2", [128, NBLK, 2], f32, isOutput=False)
    pc3t = dp("pc3t", [NBLK, 2, 256, 256], f32, isOutput=False)
    pcb3 = dp("pcb3", [128, NBLK, 2], f32, isOutput=False)
    pc4t = dp("pc4t", [NBLK, 256, LAT], f32, isOutput=False)
    pcb4 = dp("pcb4", [128, NBLK, MT], f32, isOutput=False)
    z2tt = dp("z2tt", [LAT, 64], f32, isOutput=False)
    z2tb = dp("z2tb", [64], f32, isOutput=False)
    c2pt = dp("c2pt", [3, LAT, 64], f32, isOutput=False)     # pre-scaled by 0.1
    c2pb = dp("c2pb", [64, 3], f32, isOutput=False)
    out_mlz = dp("out_mlz", [ROWS], f32, isOutput=True)      # max + logsumexp per row
    out_ll = dp("out_ll", [ROWS], f32, isOutput=True)        # label logit per row
    out_lm = dp("out_lm", [LAT, BL], f32, isOutput=True)     # latents spatial mean

    tg_in = nc.dram_tensor("tg_in", [64, NP], f32)
    if cfg.use_collective:
        tg_out = nc.dram_tensor("tg_out", [cfg.ncores, 64, NP], f32, addr_space="Shared")
    else:
        tg_out = nc.dram_tensor("tg_out", [cfg.ncores, 64, NP], f32)

    QG = min(4, BL)           # images per partition-group tile
    QUADS = [list(range(q, min(q + QG, BL))) for q in range(0, BL, QG)]
    HALVES = ((0, 4), (4, 3)) if True else None  # pr-rows split 4+3

    with tile.TileContext(nc) as tc, ExitStack() as top:
        pers = top.enter_context(tc.tile_pool(name="pers", bufs=1))
        pst = top.enter_context(tc.tile_pool(name="pst", bufs=2, space="PSUM"))

        lat = [pers.tile([128, NP], f32, tag=f"lat{m}", name=f"lat{m}") for m in range(MT)]

        # ---------------- encoder ----------------
        with ExitStack() as enc:
            ew = enc.enter_context(tc.tile_pool(name="encw", bufs=1))
            ep = enc.enter_context(tc.tile_pool(name="enc", bufs=2))
            hp = enc.enter_context(tc.tile_pool(name="hp", bufs=2))
            hcp = enc.enter_context(tc.tile_pool(name="hcp", bufs=4))
            ps1 = enc.enter_context(tc.tile_pool(name="ps1", bufs=2, space="PSUM"))
            ps2 = enc.enter_context(tc.tile_pool(name="ps2", bufs=4, space="PSUM"))

            w1_sb = ew.tile([128, 8, 256], bf16)
            for g in range(QG):
                nc.gpsimd.dma_start(
                    out=w1_sb[32 * g:32 * g + 24], in_=w1r[:, :, :]
                )
            b1_sb = ew.tile([128, 2], f32)
            nc.sync.dma_start(out=b1_sb, in_=b1p[:, :])
            w2a = ew.tile([128, LAT], f32)
            w2b = ew.tile([128, LAT], f32)
            nc.sync.dma_start(out=w2a, in_=w2t[0:128, :])
            nc.sync.dma_start(out=w2b, in_=w2t[128:256, :])
            b2d = ew.tile([128, MT], f32)
            nc.sync.dma_start(out=b2d, in_=b2[:, :])
            nc.vector.tensor_scalar_mul(b2d, b2d, 1.0 / 64.0)

            for quad in QUADS:
                for pr0, npr in HALVES:
                    # Brow: partition group 32g holds image quad[g], rows
                    # (c*8+kh); free = (pr, oh, col) raw image rows, deduped.
                    Brow = ep.tile([128, npr, 8, 256], bf16, tag="Brow")
                    # B32: per-patch expanded+masked view, free (pr,pc,oh,x)
                    B32 = ep.tile([128, npr, 7, 8, 64], bf16, tag="B32")
                    RM32 = ep.tile([128, npr, 7, 8], bf16, tag="RM32")
                    CM32 = ep.tile([128, npr, 7, 64], bf16, tag="CM32")
                    if cfg.sim_safe:
                        nc.gpsimd.memset(Brow, 0.0)
                        nc.gpsimd.memset(RM32, 0.0)
                        nc.gpsimd.memset(CM32, 0.0)
                    for g, img in enumerate(quad):
                        for c in range(3):
                            for prl in range(npr):
                                ioff = (img * 3 + c) * 65536 + (pr0 + prl) * 8192
                                nc.gpsimd.dma_start(
                                    out=Brow[32 * g + 8 * c:32 * g + 8 * c + 8, prl],
                                    in_=_apv(
                                        images[:, :, :, :],
                                        [[256, 8], [2048, 8], [1, 256]],
                                        ioff,
                                    ),
                                )
                        moff = (img * 24) * 392 + pr0 * 56
                        nc.gpsimd.dma_start(
                            out=RM32[32 * g:32 * g + 24],
                            in_=_apv(rowmx[:, :, :], [[392, 24], [1, npr * 56]], moff),
                        )
                        moff = (img * 24) * 3136 + pr0 * 448
                        nc.gpsimd.dma_start(
                            out=CM32[32 * g:32 * g + 24],
                            in_=_apv(colmx[:, :, :], [[3136, 24], [1, npr * 448]], moff),
                        )
                    # masked = (B+1)*RM*CM; pass 1 also expands rows to
                    # overlapping patches via an overlapping-stride read view.
                    # walrus limits stt to 3D -> one op per (pr, oh).
                    brf = Brow[:, :, :, :]
                    rmf = RM32[:, :, :, :]
                    cmf = CM32[:, :, :, :]
                    for prl in range(npr):
                        for oh in range(8):
                            nc.vector.scalar_tensor_tensor(
                                out=B32[:, prl, :, oh, :], scalar=1.0,
                                in0=_apv(brf, [brf.ap[0], [32, 7], [1, 64]],
                                         prl * 2048 + oh * 256),
                                in1=_apv(rmf, [rmf.ap[0], [8, 7], [0, 64]],
                                         prl * 56 + oh),
                                op0=AL.add, op1=AL.mult,
                            )
                            nc.vector.scalar_tensor_tensor(
                                out=B32[:, prl, :, oh, :], scalar=0.0,
                                in0=B32[:, prl, :, oh, :],
                                in1=_apv(cmf, [cmf.ap[0], [64, 7], [1, 64]],
                                         prl * 448),
                                op0=AL.add, op1=AL.mult,
                            )
                    for g, img in enumerate(quad):
                        for prl in range(npr):
                            pr = pr0 + prl
                            bsl = B32[32 * g:32 * g + 24]
                            h_pr = []
                            for cot in range(2):
                                ps = ps1.tile([128, 448], f32, tag="c1")
                                for kw in range(8):
                                    rhs = _apv(
                                        bsl, [bsl.ap[0], [512, 7], [64, 8], [8, 8]],
                                        prl * 3584 + kw,
                                    )
                                    nc.tensor.matmul(
                                        ps,
                                        w1_sb[32 * g:32 * g + 24, kw,
                                              cot * 128:(cot + 1) * 128],
                                        rhs, start=(kw == 0), stop=(kw == 7),
                                        tile_position=(32 * g, 0),
                                    )
                                h = hp.tile([128, 448], f32, tag=f"h{cot}")
                                nc.scalar.activation(
                                    h, ps, AF.Relu, bias=b1_sb[:, cot:cot + 1],
                                    scale=1.0,
                                )
                                h_pr.append(h)
                            for m in range(MT):
                                ps = ps2.tile([128, 448], f32, tag="c2")
                                nc.tensor.matmul(
                                    ps, w2a[:, m * 128:(m + 1) * 128], h_pr[0],
                                    start=True, stop=False,
                                )
                                nc.tensor.matmul(
                                    ps, w2b[:, m * 128:(m + 1) * 128], h_pr[1],
                                    start=False, stop=True,
                                )
                                hc = hcp.tile([128, 7, 64], bf16, tag="hc")
                                # relu(x+b2)/64 == relu(x/64 + b2/64)
                                nc.scalar.activation(
                                    hc, ps, AF.Relu, bias=b2d[:, m:m + 1],
                                    scale=1.0 / 64.0,
                                )
                                base = img * 49 + pr * 7
                                nc.vector.reduce_sum(
                                    out=lat[m][:, base:base + 7], in_=hc, axis=AX.X
                                )

        # ---------------- latents mean + targets (+ gather) ----------------
        z2t_sb = pers.tile([128, KC, 64], f32)
        nc.sync.dma_start(
            out=z2t_sb, in_=_apv(z2tt[:, :], [[64, 128], [8192, KC], [1, 64]])
        )
        z2tb_sb = pers.tile([64, 1], f32)
        nc.sync.dma_start(out=z2tb_sb, in_=_apv(z2tb[:], [[1, 64], [0, 1]]))
        for m in range(MT):
            lm = pers.tile([128, BL], f32, tag="lm", name="lm", bufs=2)
            nc.vector.reduce_sum(
                out=lm,
                in_=lat[m][:, :].rearrange("p (b s) -> p b s", s=49),
                axis=AX.X,
            )
            nc.vector.tensor_scalar_mul(lm, lm, 1.0 / 49.0)
            nc.sync.dma_start(out=out_lm[m * 128:(m + 1) * 128, :], in_=lm)

        psT = pst.tile([64, NP], f32, tag="pt")
        for kc in range(KC):
            nc.tensor.matmul(
                psT, z2t_sb[:, kc, :], lat[kc][:, :],
                start=(kc == 0), stop=(kc == KC - 1),
            )
        T_sb = pers.tile([64, NP], f32)
        nc.scalar.activation(T_sb, psT, AF.Identity, bias=z2tb_sb[:, 0:1], scale=1.0)
        nc.sync.dma_start(out=tg_in[:, :], in_=T_sb)
        if cfg.use_collective:
            nc.gpsimd.collective_compute(
                "AllGather",
                AL.bypass,
                replica_groups=[list(range(cfg.ncores))],
                ins=[tg_in[:, :]],
                outs=[tg_out[:, :, :]],
            )
        else:
            nc.gpsimd.dma_start(out=tg_out[0], in_=tg_in[:, :])

        # ---------------- pixelcnn ----------------
        x = list(lat)
        with ExitStack() as pcs:
            pw = pcs.enter_context(tc.tile_pool(name="pcw", bufs=2))
            yp = pcs.enter_context(tc.tile_pool(name="yp", bufs=2))
            ps3 = pcs.enter_context(tc.tile_pool(name="ps3", bufs=3, space="PSUM"))

            pb1_sb = pers.tile([128, NBLK, 2], f32)
            nc.sync.dma_start(
                out=pb1_sb, in_=pcb1[:, :, :]
            )
            pb2_sb = pers.tile([128, NBLK, 2], f32)
            nc.sync.dma_start(
                out=pb2_sb, in_=pcb2[:, :, :]
            )
            pb3_sb = pers.tile([128, NBLK, 2], f32)
            nc.sync.dma_start(
                out=pb3_sb, in_=pcb3[:, :, :]
            )
            pb4_sb = pers.tile([128, NBLK, MT], f32)
            nc.sync.dma_start(
                out=pb4_sb, in_=pcb4[:, :, :]
            )

            for k in range(NBLK):
                pc1_sb = pw.tile([128, KC, 256], f32, tag="pc1")
                nc.sync.dma_start(
                    out=pc1_sb,
                    in_=_apv(
                        pc1t[:, :, :], [[256, 128], [32768, KC], [1, 256]],
                        k * LAT * 256,
                    ),
                )
                pc2_sb = pw.tile([128, 3, 2, 256], f32, tag="pc2")
                for dx in range(3):
                    nc.sync.dma_start(
                        out=pc2_sb[:, dx],
                        in_=_apv(
                            pc2t[:, :, :, :],
                            [[256, 128], [32768, 2], [1, 256]],
                            (k * 3 + dx) * 65536,
                        ),
                    )
                pc3_sb = pw.tile([128, 2, 2, 256], f32, tag="pc3")
                for dy in range(2):
                    nc.sync.dma_start(
                        out=pc3_sb[:, dy],
                        in_=_apv(
                            pc3t[:, :, :, :],
                            [[256, 128], [32768, 2], [1, 256]],
                            (k * 2 + dy) * 65536,
                        ),
                    )
                pc4_sb = pw.tile([128, 2, LAT], f32, tag="pc4")
                nc.sync.dma_start(
                    out=pc4_sb,
                    in_=_apv(
                        pc4t[:, :, :], [[LAT, 128], [128 * LAT, 2], [1, LAT]],
                        k * 256 * LAT,
                    ),
                )

                # y1 = relu(1x1 conv LAT->256), written into col-padded buffer
                y1p = []
                for m2 in range(2):
                    ps = ps3.tile([128, NP], f32, tag="py")
                    for kc in range(KC):
                        nc.tensor.matmul(
                            ps, pc1_sb[:, kc, m2 * 128:(m2 + 1) * 128], x[kc][:, :],
                            start=(kc == 0), stop=(kc == KC - 1),
                        )
                    t = yp.tile([128, BL, 7, 9], f32, tag=f"y1p{m2}")
                    nc.vector.memset(t, 0.0)
                    nc.scalar.activation(
                        t[:, :, :, 1:8], ps, AF.Relu,
                        bias=pb1_sb[:, k, m2:m2 + 1], scale=1.0,
                    )
                    y1p.append(t)
                # y2 = relu(1x3 conv along columns), row-padded buffer
                y2p = []
                for m2 in range(2):
                    ps = ps3.tile([128, NP], f32, tag="py")
                    first = True
                    for dx in range(3):
                        for kc in range(2):
                            nc.tensor.matmul(
                                ps, pc2_sb[:, dx, kc, m2 * 128:(m2 + 1) * 128],
                                y1p[kc][:, :, :, dx:dx + 7],
                                start=first, stop=(dx == 2 and kc == 1),
                            )
                            first = False
                    t = yp.tile([128, BL, 8, 7], f32, tag=f"y2p{m2}")
                    nc.vector.memset(t, 0.0)
                    nc.scalar.activation(
                        t[:, :, 1:8, :], ps, AF.Relu,
                        bias=pb2_sb[:, k, m2:m2 + 1], scale=1.0,
                    )
                    y2p.append(t)
                # y3 = relu(2x1 conv along rows, top pad)
                y3 = []
                for m2 in range(2):
                    ps = ps3.tile([128, NP], f32, tag="py")
                    first = True
                    for dy in range(2):
                        for kc in range(2):
                            nc.tensor.matmul(
                                ps, pc3_sb[:, dy, kc, m2 * 128:(m2 + 1) * 128],
                                y2p[kc][:, :, dy:dy + 7, :],
                                start=first, stop=(dy == 1 and kc == 1),
                            )
                            first = False
                    t = yp.tile([128, NP], f32, tag=f"y3{m2}")
                    nc.scalar.activation(
                        t, ps, AF.Relu, bias=pb3_sb[:, k, m2:m2 + 1], scale=1.0
                    )
                    y3.append(t)
                # y4 = 1x1 conv 256->LAT; x = relu(y4 + b4 + x)
                for m in range(MT):
                    ps = ps3.tile([128, NP], f32, tag="py4")
                    for kc in range(2):
                        nc.tensor.matmul(
                            ps, pc4_sb[:, kc, m * 128:(m + 1) * 128], y3[kc][:, :],
                            start=(kc == 0), stop=(kc == 1),
                        )
                    t = yp.tile([128, NP], f32, tag="resid")
                    nc.vector.scalar_tensor_tensor(
                        out=t, in0=ps, scalar=pb4_sb[:, k, m:m + 1], in1=x[m][:, :],
                        op0=AL.add, op1=AL.add,
                    )
                    nc.scalar.activation(x[m][:, :], t, AF.Relu)

        # ---------------- preds + logits + loss pieces ----------------
        with ExitStack() as lgs:
            lp = lgs.enter_context(tc.tile_pool(name="lp", bufs=2))
            lp1 = lgs.enter_context(tc.tile_pool(name="lp1", bufs=1))
            ps4 = lgs.enter_context(tc.tile_pool(name="ps4", bufs=2, space="PSUM"))

            c2p_sb = lp1.tile([128, 3, KC, 64], f32)
            for si in range(3):
                nc.sync.dma_start(
                    out=c2p_sb[:, si],
                    in_=_apv(c2pt[:, :, :], [[64, 128], [8192, KC], [1, 64]],
                             si * LAT * 64),
                )
            c2pb_sb = lp1.tile([64, 3], f32)
            nc.sync.dma_start(out=c2pb_sb, in_=c2pb[:, :])

            preds_cat = lp1.tile([64, ROWS], f32)
            for si in range(3):
                psP = ps4.tile([64, NP], f32, tag="pp")
                for kc in range(KC):
                    nc.tensor.matmul(
                        psP, c2p_sb[:, si, kc, :], x[kc][:, :],
                        start=(kc == 0), stop=(kc == KC - 1),
                    )
                P_sb = lp.tile([64, BL, 7, 7], f32, tag="P")
                nc.scalar.activation(
                    P_sb, psP, AF.Identity, bias=c2pb_sb[:, si:si + 1], scale=1.0
                )
                n_i = BL * HI[si] * 7
                nc.vector.tensor_copy(
                    preds_cat[:, OFFS[si]:OFFS[si] + n_i],
                    P_sb[:, :, 0:HI[si], :],
                )
            # label logits: preds . target[label]; labels are same-image patches
            prod = lp1.tile([64, ROWS], f32)
            Tr = T_sb[:, :].rearrange("t (b r c) -> t b r c", r=7, c=7)
            for si in range(3):
                n_i = BL * HI[si] * 7
                nc.vector.tensor_mul(
                    prod[:, OFFS[si]:OFFS[si] + n_i],
                    preds_cat[:, OFFS[si]:OFFS[si] + n_i],
                    Tr[:, :, si + 3:si + 3 + HI[si], :],
                )
            ones_sb = lp1.tile([64, 1], f32)
            nc.vector.memset(ones_sb, 1.0)
            psL = ps4.tile([1, ROWS], f32, tag="pl")
            nc.tensor.matmul(psL, ones_sb, prod, start=True, stop=True)
            ll_sb = lp1.tile([1, ROWS], f32)
            nc.scalar.copy(ll_sb, psL)
            nc.sync.dma_start(out=out_ll[:], in_=ll_sb)

            # logits against all gathered targets, streamed row-block-wise
            Tfull = lp1.tile([64, cfg.ncores, NP], f32)
            nc.sync.dma_start(
                out=Tfull,
                in_=tg_out[:, :, :].rearrange("c t p -> t c p"),
            )
            tf = Tfull[:, :, :]
            tflat = bass.AP(tensor=tf.tensor, offset=tf.offset, ap=[tf.ap[0], [1, GC]])
            n_mb = 4 if ROWS % 4 == 0 and ROWS // 4 <= 128 else 1
            MB = ROWS // n_mb
            csz = 448
            n_ch = (GC + csz - 1) // csz
            for m4 in range(n_mb):
                lg = lp.tile([MB, GC], f32, tag="lg")
                for nch in range(n_ch):
                    w = min(csz, GC - nch * csz)
                    ps = ps4.tile([MB, csz], f32, tag="plg")
                    nc.tensor.matmul(
                        ps[:, 0:w],
                        preds_cat[:, m4 * MB:(m4 + 1) * MB],
                        _apv(tflat, [tflat.ap[0], [1, w]], nch * csz),
                        start=True, stop=True,
                    )
                    nc.scalar.copy(lg[:, nch * csz:nch * csz + w], ps[:, 0:w])
                mx = lp.tile([MB, 1], f32, tag="mx")
                nc.vector.reduce_max(out=mx, in_=lg, axis=AX.X)
                nmx = lp.tile([MB, 1], f32, tag="nmx")
                nc.vector.tensor_scalar_mul(nmx, mx, -1.0)
                ex = lp.tile([MB, GC], f32, tag="ex")
                nc.scalar.activation(ex, lg, AF.Exp, bias=nmx, scale=1.0)
                zz = lp.tile([MB, 1], f32, tag="zz")
                nc.vector.reduce_sum(out=zz, in_=ex, axis=AX.X)
                lnz = lp.tile([MB, 1], f32, tag="lnz")
                nc.scalar.activation(lnz, zz, AF.Ln)
                mlz = lp.tile([MB, 1], f32, tag="mlz")
                nc.vector.tensor_add(mlz, mx, lnz)
                nc.sync.dma_start(out=out_mlz[m4 * MB:(m4 + 1) * MB], in_=mlz)

    return nc


# ---------------------------------------------------------------------------


def host_prep(inputs, cfg: Cfg):
    """Per-core input maps. Only sharding, layout transforms of weights, and
    index-arithmetic mask vectors happen here — all FLOPs stay on device."""
    LAT, NBLK = cfg.latent, cfg.nblk
    EMB_SCALE = 0.1
    images = np.ascontiguousarray(inputs["images"], dtype=np.float32)
    rnd = np.asarray(inputs["rnd"]).astype(np.int32)
    B = images.shape[0]

    r0 = (rnd // 4).astype(np.int64)
    c0 = (rnd % 4).astype(np.int64)
    idx = np.arange(64)
    rowm_all = ((idx[None, :] >= r0[:, None]) & (idx[None, :] < r0[:, None] + 60)).astype(np.float32)
    colm_all = ((idx[None, :] >= c0[:, None]) & (idx[None, :] < c0[:, None] + 60)).astype(np.float32)
    # device layouts: rowmx[b, c*8+kh, (pr,pc,oh)] = rowm[p, 8*oh+kh]
    #                 colmx[b, c*8+kh, (pr,pc,x)]  = colm[p, x]
    rm_p = rowm_all.reshape(B, 49, 8, 8)                       # (b, p, oh, kh)
    rowmx_all = np.broadcast_to(
        rm_p.transpose(0, 3, 1, 2)[:, None, :, :, :], (B, 3, 8, 49, 8)
    ).reshape(B, 24, 392).astype(np.float32)
    cm_p = colm_all.reshape(B, 49, 64)
    colmx_all = np.broadcast_to(
        cm_p[:, None, None, :, :], (B, 3, 8, 49, 64)
    ).reshape(B, 24, 3136).astype(np.float32)

    w1 = np.asarray(inputs["enc_w1"], dtype=np.float32)          # (256,3,8,8)
    w1r = np.ascontiguousarray(w1.transpose(1, 2, 3, 0).reshape(24, 8, 256))
    b1p = (np.asarray(inputs["enc_b1"], np.float32) - w1.sum(axis=(1, 2, 3)))
    w2t = np.ascontiguousarray(np.asarray(inputs["enc_w2"], np.float32)[:, :, 0, 0].T)  # (256,LAT)
    b2 = np.asarray(inputs["enc_b2"], np.float32)
    pc1t = np.ascontiguousarray(np.asarray(inputs["pc_w1"], np.float32)[:, :, :, 0, 0].transpose(0, 2, 1))  # (5,LAT,256)
    pc2t = np.ascontiguousarray(np.asarray(inputs["pc_w2"], np.float32)[:, :, :, 0, :].transpose(0, 3, 2, 1))  # (5,3,256in,256out)
    pc3t = np.ascontiguousarray(np.asarray(inputs["pc_w3"], np.float32)[:, :, :, :, 0].transpose(0, 3, 2, 1))  # (5,2,256in,256out)
    pc4t = np.ascontiguousarray(np.asarray(inputs["pc_w4"], np.float32)[:, :, :, 0, 0].transpose(0, 2, 1))  # (5,256,LAT)
    z2tt = np.ascontiguousarray(np.asarray(inputs["z2t_w"], np.float32)[:, :, 0, 0].T)  # (LAT,64)
    c2pt = np.ascontiguousarray(np.asarray(inputs["c2p_w"], np.float32)[:, :, :, 0, 0].transpose(0, 2, 1)) * EMB_SCALE
    c2pb = np.asarray(inputs["c2p_b"], np.float32) * EMB_SCALE

    MT = cfg.mt
    mkb = lambda v, n: np.ascontiguousarray(
        np.asarray(v, np.float32).reshape(NBLK, n, 128).transpose(2, 0, 1)
    )
    shared = {
        "w1r": w1r,
        "b1p": np.ascontiguousarray(b1p.reshape(2, 128).T),
        "w2t": w2t,
        "b2": np.ascontiguousarray(b2.reshape(MT, 128).T),
        "pc1t": pc1t, "pcb1": mkb(inputs["pc_b1"], 2),
        "pc2t": pc2t, "pcb2": mkb(inputs["pc_b2"], 2),
        "pc3t": pc3t, "pcb3": mkb(inputs["pc_b3"], 2),
        "pc4t": pc4t, "pcb4": mkb(inputs["pc_b4"], MT),
        "z2tt": z2tt, "z2tb": np.asarray(inputs["z2t_b"], np.float32),
        "c2pt": c2pt, "c2pb": np.ascontiguousarray(c2pb.T),
    }
    in_maps = []
    for c in range(cfg.ncores):
        bsl = slice(c * cfg.bl, (c + 1) * cfg.bl)
        m = dict(shared)
        m["images"] = np.ascontiguousarray(images[bsl])
        m["rowmx"] = np.ascontiguousarray(rowmx_all[bsl])
        m["colmx"] = np.ascontiguousarray(colmx_all[bsl])
        in_maps.append(m)
    return in_maps


def host_epilogue(results, cfg: Cfg):
    B, BL = cfg.ncores * cfg.bl, cfg.bl
    HI = (4, 3, 2)
    OFFS = (0, BL * 28, BL * 49, BL * 63)
    loss = np.float64(0.0)
    for si in range(3):
        tot = B * HI[si] * 7
        s = np.float64(0.0)
        for r in results:
            sl = slice(OFFS[si], OFFS[si + 1])
            s += (r["out_mlz"][sl].astype(np.float64) - r["out_ll"][sl].astype(np.float64)).sum()
        loss += s / tot
    latmean = np.concatenate([r["out_lm"].T for r in results], axis=0)  # (B, LAT)
    return np.float32(loss), latmean.astype(np.float32)


_CACHE = {}


def kernel(**inputs):
    cfg = Cfg()
    if "nc" not in _CACHE:
        _CACHE["nc"] = build_nc(cfg)
    nc = _CACHE["nc"]
    in_maps = host_prep(inputs, cfg)
    from concourse.bass_utils import run_bass_kernel_spmd

    if not _CACHE.get("split_done"):
        _split_multiwait(nc)
        _CACHE["split_done"] = True
    res = run_bass_kernel_spmd(nc, in_maps, list(range(cfg.ncores)))
    return host_epilogue(res.results, cfg)


# revision 42
# speedup vs baseline: 2.2893x; 2.2893x over previous
"""CPC (contrastive predictive coding) forward pass on 8 Trainium2 NeuronCores.

Data-parallel over the batch: each core processes 8 images (392 patches).
Single SPMD launch; the contrastive targets are exchanged with an on-device
AllGather so each core can normalize its logits over all 3136 negatives.

Encoder conv1 (3->256, k8 s8) runs as 8 PSUM-accumulated matmuls over the
kernel-column index kw with K=(channel, kernel-row)=24 on the partitions and
strided free-dim slices of the patch-row tensor B[(c,kh), (patch,oh,x)] —
this keeps every DMA from HBM at 256B-contiguous runs (no im2col gather).
Border masking is (p+1)*rowmask*colmask - 1 folded into conv1's bias.
"""

import sys
from contextlib import ExitStack
from dataclasses import dataclass

import numpy as np

for _p in ("/opt/trn_rl_repo", "/root/.axon_site/_ro/trn_rl_repo"):
    if _p not in sys.path:
        sys.path.append(_p)

import concourse.bass as bass
import concourse.mybir as mybir
import concourse.tile as tile

f32 = mybir.dt.float32
f32r = mybir.dt.float32r
bf16 = mybir.dt.bfloat16
AF = mybir.ActivationFunctionType
AL = mybir.AluOpType
AX = mybir.AxisListType


@dataclass(frozen=True)
class Cfg:
    ncores: int = 8
    bl: int = 8          # images per core
    latent: int = 2048
    nblk: int = 5        # pixelcnn residual blocks
    use_collective: bool = True
    sim_safe: bool = False   # memset pad partitions so CoreSim sees no uninit reads

    @property
    def mt(self):        # latent 128-tiles
        return self.latent // 128

    @property
    def np_(self):       # patches per core
        return self.bl * 49

    @property
    def rows(self):      # contrastive rows per core (steps 2,3,4 kept rows)
        return self.bl * (4 + 3 + 2) * 7

    @property
    def gcols(self):     # global negatives
        return self.ncores * self.np_


def _apv(base, dims, extra_off=0):
    return bass.AP(tensor=base.tensor, offset=base.offset + extra_off, ap=list(dims))


def _split_multiwait(nc):
    """This env's walrus accepts only one sync-wait per instruction; split
    extras onto single-wait NoOps placed just before."""
    for fn in nc.m.functions:
        for blk in fn.blocks:
            new_insts = []
            for inst in blk.instructions:
                si = inst.sync_info
                if si is not None and len(si.on_wait) > 1:
                    waits = list(si.on_wait)
                    for j, w in enumerate(waits[:-1]):
                        new_insts.append(
                            mybir.InstNoOp(
                                name=f"{inst.name}-wsplit{j}",
                                sync_info=mybir.SyncInfo(on_wait=[w], on_update=[]),
                                bass_nofuse=True,
                                engine=inst.engine,
                            )
                        )
                    inst.sync_info = mybir.SyncInfo(
                        on_wait=[waits[-1]], on_update=list(si.on_update)
                    )
                new_insts.append(inst)
            blk.instructions = new_insts


def _mmr(nc, out, lhsT, rhs, **kw):
    """Matmul on float32r-typed operands (full-rate fp32). The f32r path
    streams element pairs, so odd free sizes fall back to plain fp32."""
    if rhs.free_size() % 2:
        nc.tensor.matmul(out, lhsT.bitcast(f32), rhs.bitcast(f32), **kw)
    else:
        nc.tensor.matmul(out, lhsT, rhs, **kw)


def build_nc(cfg: Cfg):
    BL, MT, NBLK, NP = cfg.bl, cfg.mt, cfg.nblk, cfg.np_
    LAT = cfg.latent
    KC = MT  # 128-chunks of the latent contraction
    ROWS = cfg.rows
    GC = cfg.gcols
    HI = (4, 3, 2)
    OFFS = (0, BL * 28, BL * 49)  # row offsets of the 3 steps in preds_cat

    nc = bass.Bass()
    dp = nc.declare_dram_parameter
    images = dp("images", [BL, 3, 256, 256], bf16, isOutput=False)
    rowmx = dp("rowmx", [BL, 24, 392], bf16, isOutput=False)   # (b,(c,kh),(pr,pc,oh))
    colmx = dp("colmx", [BL, 24, 3136], bf16, isOutput=False)  # (b,(c,kh),(pr,pc,x))
    w1r = dp("w1r", [24, 8, 256], bf16, isOutput=False)      # (c*8+kh, kw, co)
    b1p = dp("b1p", [128, 2], f32, isOutput=False)           # b1 - sum(W1)
    w2t = dp("w2t", [256, LAT], f32r, isOutput=False)
    b2 = dp("b2", [128, MT], f32, isOutput=False)
    pc1t = dp("pc1t", [NBLK, LAT, 256], f32r, isOutput=False)
    pcb1 = dp("pcb1", [128, NBLK, 2], f32, isOutput=False)
    pc2t = dp("pc2t", [NBLK, 3, 256, 256], f32r, isOutput=False)
    pcb2 = dp("pcb2", [128, NBLK, 2], f32, isOutput=False)
    pc3t = dp("pc3t", [NBLK, 2, 256, 256], f32r, isOutput=False)
    pcb3 = dp("pcb3", [128, NBLK, 2], f32, isOutput=False)
    pc4t = dp("pc4t", [NBLK, 256, LAT], f32r, isOutput=False)
    pcb4 = dp("pcb4", [128, NBLK, MT], f32, isOutput=False)
    z2tt = dp("z2tt", [LAT, 64], f32r, isOutput=False)
    z2tb = dp("z2tb", [64], f32, isOutput=False)
    c2pt = dp("c2pt", [3, LAT, 64], f32r, isOutput=False)     # pre-scaled by 0.1
    c2pb = dp("c2pb", [64, 3], f32, isOutput=False)
    out_mlz = dp("out_mlz", [ROWS], f32, isOutput=True)      # max + logsumexp per row
    out_ll = dp("out_ll", [ROWS], f32, isOutput=True)        # label logit per row
    out_lm = dp("out_lm", [LAT, BL], f32, isOutput=True)     # latents spatial mean

    tg_in = nc.dram_tensor("tg_in", [64, NP], f32r)
    if cfg.use_collective:
        tg_out = nc.dram_tensor("tg_out", [cfg.ncores, 64, NP], f32r, addr_space="Shared")
    else:
        tg_out = nc.dram_tensor("tg_out", [cfg.ncores, 64, NP], f32r)

    QG = min(4, BL)           # images per partition-group tile
    QUADS = [list(range(q, min(q + QG, BL))) for q in range(0, BL, QG)]
    HALVES = ((0, 4), (4, 3)) if True else None  # pr-rows split 4+3

    with tile.TileContext(nc) as tc, ExitStack() as top:
        pers = top.enter_context(tc.tile_pool(name="pers", bufs=1))
        pst = top.enter_context(tc.tile_pool(name="pst", bufs=1, space="PSUM"))

        lat = [pers.tile([128, NP], f32r, tag=f"lat{m}", name=f"lat{m}") for m in range(MT)]
        latb = [pers.tile([128, NP], bf16, tag=f"latb{m}", name=f"latb{m}") for m in range(MT)]

        # ---------------- encoder ----------------
        with ExitStack() as enc:
            ew = enc.enter_context(tc.tile_pool(name="encw", bufs=1))
            ep = enc.enter_context(tc.tile_pool(name="enc", bufs=2))
            hp = enc.enter_context(tc.tile_pool(name="hp", bufs=2))
            hcp = enc.enter_context(tc.tile_pool(name="hcp", bufs=6))
            ps1 = enc.enter_context(tc.tile_pool(name="ps1", bufs=3, space="PSUM"))
            ps2 = enc.enter_context(tc.tile_pool(name="ps2", bufs=2, space="PSUM"))

            w1_sb = ew.tile([128, 8, 256], bf16)
            for g in range(QG):
                nc.sync.dma_start(
                    out=w1_sb[32 * g:32 * g + 24], in_=w1r[:, :, :]
                )
            b1d = ew.tile([128, 2], f32)
            nc.sync.dma_start(out=b1d, in_=b1p[:, :])
            nc.vector.tensor_scalar_mul(b1d, b1d, 1.0 / 64.0)
            w2a = ew.tile([128, LAT], f32r)
            w2b = ew.tile([128, LAT], f32r)
            nc.sync.dma_start(out=w2a, in_=w2t[0:128, :])
            nc.sync.dma_start(out=w2b, in_=w2t[128:256, :])
            b2d = ew.tile([128, MT], f32)
            nc.sync.dma_start(out=b2d, in_=b2[:, :])
            nc.vector.tensor_scalar_mul(b2d, b2d, 1.0 / 64.0)

            for quad in QUADS:
                for pr0, npr in HALVES:
                    # Brow: partition group 32g holds image quad[g], rows
                    # (c*8+kh); free = (pr, oh, col) raw image rows, deduped.
                    Brow = ep.tile([128, npr, 8, 256], bf16, tag="Brow")
                    # B32: per-patch expanded+masked view, free (pr,pc,oh,x)
                    B32 = ep.tile([128, npr, 7, 8, 64], bf16, tag="B32")
                    RM32 = ep.tile([128, npr, 7, 8], bf16, tag="RM32")
                    CM32 = ep.tile([128, npr, 7, 64], bf16, tag="CM32")
                    if cfg.sim_safe:
                        nc.gpsimd.memset(Brow, 0.0)
                        nc.gpsimd.memset(RM32, 0.0)
                        nc.gpsimd.memset(CM32, 0.0)
                    for g, img in enumerate(quad):
                        for c in range(3):
                            for prl in range(npr):
                                ioff = (img * 3 + c) * 65536 + (pr0 + prl) * 8192
                                deng = nc.sync if (c + prl) % 2 == 0 else nc.scalar
                                deng.dma_start(
                                    out=Brow[32 * g + 8 * c:32 * g + 8 * c + 8, prl],
                                    in_=_apv(
                                        images[:, :, :, :],
                                        [[256, 8], [2048, 8], [1, 256]],
                                        ioff,
                                    ),
                                )
                        moff = (img * 24) * 392 + pr0 * 56
                        nc.scalar.dma_start(
                            out=RM32[32 * g:32 * g + 24],
                            in_=_apv(rowmx[:, :, :], [[392, 24], [1, npr * 56]], moff),
                        )
                        moff = (img * 24) * 3136 + pr0 * 448
                        nc.scalar.dma_start(
                            out=CM32[32 * g:32 * g + 24],
                            in_=_apv(colmx[:, :, :], [[3136, 24], [1, npr * 448]], moff),
                        )
                    # masked = (B+1)*RM*CM; pass 1 also expands rows to
                    # overlapping patches via an overlapping-stride read view.
                    # walrus limits stt to 3D -> one op per (pr, oh).
                    brf = Brow[:, :, :, :]
                    rmf = RM32[:, :, :, :]
                    cmf = CM32[:, :, :, :]
                    for prl in range(npr):
                        for oh in range(8):
                            nc.vector.scalar_tensor_tensor(
                                out=B32[:, prl, :, oh, :], scalar=1.0,
                                in0=_apv(brf, [brf.ap[0], [32, 7], [1, 64]],
                                         prl * 2048 + oh * 256),
                                in1=_apv(rmf, [rmf.ap[0], [8, 7], [0, 64]],
                                         prl * 56 + oh),
                                op0=AL.add, op1=AL.mult,
                            )
                            nc.vector.scalar_tensor_tensor(
                                out=B32[:, prl, :, oh, :], scalar=0.0,
                                in0=B32[:, prl, :, oh, :],
                                in1=_apv(cmf, [cmf.ap[0], [64, 7], [1, 64]],
                                         prl * 448),
                                op0=AL.add, op1=AL.mult,
                            )
                    pairs = [(0, 1), (2, 3)] if npr == 4 else [(0, 1), (2,)]
                    for g, img in enumerate(quad):
                        bsl = B32[32 * g:32 * g + 24]
                        for pair in pairs:
                            # conv1 for the pr-pair; h pre-scaled by 1/64 so
                            # the conv2 epilogue is a plain bias+relu
                            h_pp = {}
                            for j, prl in enumerate(pair):
                                for cot in range(2):
                                    ps = ps1.tile([128, 448], f32, tag="c1")
                                    for kw in range(8):
                                        rhs = _apv(
                                            bsl,
                                            [bsl.ap[0], [512, 7], [64, 8], [8, 8]],
                                            prl * 3584 + kw,
                                        )
                                        nc.tensor.matmul(
                                            ps,
                                            w1_sb[32 * g:32 * g + 24, kw,
                                                  cot * 128:(cot + 1) * 128],
                                            rhs, start=(kw == 0), stop=(kw == 7),
                                            tile_position=(32 * g, 0),
                                        )
                                    h = hp.tile([128, 448], f32r, tag=f"h{j}{cot}")
                                    nc.scalar.activation(
                                        h, ps, AF.Relu, bias=b1d[:, cot:cot + 1],
                                        scale=1.0 / 64.0,
                                    )
                                    h_pp[(j, cot)] = h
                            base = img * 49 + (pr0 + pair[0]) * 7
                            w = 7 * len(pair)
                            for m in range(MT):
                                ps = ps2.tile([128, 2, 512], f32, tag="c2")
                                for j, prl in enumerate(pair):
                                    _mmr(nc,
                                        ps[:, j, 0:448],
                                        w2a[:, m * 128:(m + 1) * 128], h_pp[(j, 0)],
                                        start=True, stop=False,
                                    )
                                    _mmr(nc,
                                        ps[:, j, 0:448],
                                        w2b[:, m * 128:(m + 1) * 128], h_pp[(j, 1)],
                                        start=False, stop=True,
                                    )
                                psf = ps[:, :, :]
                                psv = _apv(psf, [psf.ap[0], [512, len(pair)], [1, 448]])
                                hc = hcp.tile([128, 2, 7, 64], bf16, tag="hc")
                                hcf = hc[:, :, :, :]
                                hcv = _apv(hcf, [hcf.ap[0], [448, len(pair)], [1, 448]])
                                eng = nc.scalar
                                if eng is nc.scalar:
                                    nc.scalar.activation(
                                        hcv, psv, AF.Relu, bias=b2d[:, m:m + 1],
                                        scale=1.0,
                                    )
                                else:
                                    eng.tensor_scalar(
                                        hcv, psv, b2d[:, m:m + 1], 0.0,
                                        op0=AL.add, op1=AL.max,
                                    )
                                with nc.allow_low_precision(
                                    reason="64-elem pool; bf16 out is plenty"
                                ):
                                    nc.vector.reduce_sum(
                                        out=latb[m][:, base:base + w],
                                    in_=_apv(hcf, [hcf.ap[0], [64, w], [1, 64]]),
                                    axis=AX.X,
                                )

        for m in range(MT):
            nc.vector.tensor_copy(lat[m][:, :], latb[m][:, :])

        # ---------------- latents mean + targets (+ gather) ----------------
        z2t_sb = pers.tile([128, KC, 64], f32r)
        nc.sync.dma_start(
            out=z2t_sb, in_=_apv(z2tt[:, :], [[64, 128], [8192, KC], [1, 64]])
        )
        z2tb_sb = pers.tile([64, 1], f32)
        nc.sync.dma_start(out=z2tb_sb, in_=_apv(z2tb[:], [[1, 64], [0, 1]]))
        for m in range(MT):
            lm = pers.tile([128, BL], f32, tag="lm", name="lm", bufs=2)
            nc.vector.reduce_sum(
                out=lm,
                in_=lat[m][:, :].rearrange("p (b s) -> p b s", s=49),
                axis=AX.X,
            )
            nc.vector.tensor_scalar_mul(lm, lm, 1.0 / 49.0)
            nc.sync.dma_start(out=out_lm[m * 128:(m + 1) * 128, :], in_=lm)

        psT = pst.tile([64, NP], f32, tag="pt")
        for kc in range(KC):
            _mmr(nc,
                psT, z2t_sb[:, kc, :], lat[kc][:, :],
                start=(kc == 0), stop=(kc == KC - 1),
            )
        T_sb = pers.tile([64, NP], f32r)
        nc.scalar.activation(T_sb, psT, AF.Identity, bias=z2tb_sb[:, 0:1], scale=1.0)
        nc.sync.dma_start(out=tg_in[:, :], in_=T_sb)
        if cfg.use_collective:
            nc.gpsimd.collective_compute(
                "AllGather",
                AL.bypass,
                replica_groups=[list(range(cfg.ncores))],
                ins=[tg_in[:, :]],
                outs=[tg_out[:, :, :]],
            )
        else:
            nc.gpsimd.dma_start(out=tg_out[0], in_=tg_in[:, :])

        # ---------------- pixelcnn ----------------
        x = list(lat)
        with ExitStack() as pcs:
            pw = pcs.enter_context(tc.tile_pool(name="pcw", bufs=2))
            yp = pcs.enter_context(tc.tile_pool(name="yp", bufs=2))
            ps3 = pcs.enter_context(tc.tile_pool(name="ps3", bufs=3, space="PSUM"))

            pb1_sb = pers.tile([128, NBLK, 2], f32)
            nc.sync.dma_start(
                out=pb1_sb, in_=pcb1[:, :, :]
            )
            pb2_sb = pers.tile([128, NBLK, 2], f32)
            nc.sync.dma_start(
                out=pb2_sb, in_=pcb2[:, :, :]
            )
            pb3_sb = pers.tile([128, NBLK, 2], f32)
            nc.sync.dma_start(
                out=pb3_sb, in_=pcb3[:, :, :]
            )
            pb4_sb = pers.tile([128, NBLK, MT], f32)
            nc.sync.dma_start(
                out=pb4_sb, in_=pcb4[:, :, :]
            )

            for k in range(NBLK):
                pc1_sb = pw.tile([128, KC, 256], f32r, tag="pc1")
                nc.sync.dma_start(
                    out=pc1_sb,
                    in_=_apv(
                        pc1t[:, :, :], [[256, 128], [32768, KC], [1, 256]],
                        k * LAT * 256,
                    ),
                )
                pc2_sb = pw.tile([128, 3, 2, 256], f32r, tag="pc2")
                for dx in range(3):
                    nc.sync.dma_start(
                        out=pc2_sb[:, dx],
                        in_=_apv(
                            pc2t[:, :, :, :],
                            [[256, 128], [32768, 2], [1, 256]],
                            (k * 3 + dx) * 65536,
                        ),
                    )
                pc3_sb = pw.tile([128, 2, 2, 256], f32r, tag="pc3")
                for dy in range(2):
                    nc.sync.dma_start(
                        out=pc3_sb[:, dy],
                        in_=_apv(
                            pc3t[:, :, :, :],
                            [[256, 128], [32768, 2], [1, 256]],
                            (k * 2 + dy) * 65536,
                        ),
                    )
                pc4_sb = pw.tile([128, 2, LAT], f32r, tag="pc4")
                nc.sync.dma_start(
                    out=pc4_sb,
                    in_=_apv(
                        pc4t[:, :, :], [[LAT, 128], [128 * LAT, 2], [1, LAT]],
                        k * 256 * LAT,
                    ),
                )

                # y1 = relu(1x1 conv LAT->256), written into col-padded buffer
                y1p = []
                for m2 in range(2):
                    ps = ps3.tile([128, NP], f32, tag="py")
                    for kc in range(KC):
                        _mmr(nc,
                            ps, pc1_sb[:, kc, m2 * 128:(m2 + 1) * 128], x[kc][:, :],
                            start=(kc == 0), stop=(kc == KC - 1),
                        )
                    t = yp.tile([128, BL, 7, 9], f32r, tag=f"y1p{m2}")
                    nc.vector.memset(t[:, :, :, :].bitcast(f32), 0.0)
                    nc.scalar.activation(
                        t[:, :, :, 1:8], ps, AF.Relu,
                        bias=pb1_sb[:, k, m2:m2 + 1], scale=1.0,
                    )
                    y1p.append(t)
                # y2 = relu(1x3 conv along columns), row-padded buffer
                y2p = []
                for m2 in range(2):
                    ps = ps3.tile([128, NP], f32, tag="py")
                    first = True
                    for dx in range(3):
                        for kc in range(2):
                            nc.tensor.matmul(
                                ps,
                                pc2_sb[:, dx, kc, m2 * 128:(m2 + 1) * 128].bitcast(f32),
                                y1p[kc][:, :, :, dx:dx + 7].bitcast(f32),
                                start=first, stop=(dx == 2 and kc == 1),
                            )
                            first = False
                    t = yp.tile([128, BL, 8, 7], f32r, tag=f"y2p{m2}")
                    nc.vector.memset(t[:, :, :, :].bitcast(f32), 0.0)
                    nc.scalar.activation(
                        t[:, :, 1:8, :], ps, AF.Relu,
                        bias=pb2_sb[:, k, m2:m2 + 1], scale=1.0,
                    )
                    y2p.append(t)
                # y3 = relu(2x1 conv along rows, top pad)
                y3 = []
                for m2 in range(2):
                    ps = ps3.tile([128, NP], f32, tag="py")
                    first = True
                    for dy in range(2):
                        for kc in range(2):
                            nc.tensor.matmul(
                                ps,
                                pc3_sb[:, dy, kc, m2 * 128:(m2 + 1) * 128].bitcast(f32),
                                y2p[kc][:, :, dy:dy + 7, :].bitcast(f32),
                                start=first, stop=(dy == 1 and kc == 1),
                            )
                            first = False
                    t = yp.tile([128, NP], f32r, tag=f"y3{m2}")
                    nc.scalar.activation(
                        t, ps, AF.Relu, bias=pb3_sb[:, k, m2:m2 + 1], scale=1.0
                    )
                    y3.append(t)
                # y4 = 1x1 conv 256->LAT; x = relu(y4 + b4 + x)
                for m in range(MT):
                    ps = ps3.tile([128, NP], f32, tag="py4")
                    for kc in range(2):
                        _mmr(nc,
                            ps, pc4_sb[:, kc, m * 128:(m + 1) * 128], y3[kc][:, :],
                            start=(kc == 0), stop=(kc == 1),
                        )
                    t = yp.tile([128, NP], f32, tag="resid")
                    nc.vector.scalar_tensor_tensor(
                        out=t, in0=ps, scalar=pb4_sb[:, k, m:m + 1], in1=x[m][:, :],
                        op0=AL.add, op1=AL.add,
                    )
                    nc.scalar.activation(x[m][:, :], t, AF.Relu)

        # ---------------- preds + logits + loss pieces ----------------
        with ExitStack() as lgs:
            lp = lgs.enter_context(tc.tile_pool(name="lp", bufs=2))
            lp1 = lgs.enter_context(tc.tile_pool(name="lp1", bufs=1))
            ps4 = lgs.enter_context(tc.tile_pool(name="ps4", bufs=2, space="PSUM"))

            c2p_sb = lp1.tile([128, 3, KC, 64], f32r)
            for si in range(3):
                nc.sync.dma_start(
                    out=c2p_sb[:, si],
                    in_=_apv(c2pt[:, :, :], [[64, 128], [8192, KC], [1, 64]],
                             si * LAT * 64),
                )
            c2pb_sb = lp1.tile([64, 3], f32)
            nc.sync.dma_start(out=c2pb_sb, in_=c2pb[:, :])

            preds_cat = lp1.tile([64, ROWS], f32r)
            for si in range(3):
                psP = ps4.tile([64, NP], f32, tag="pp")
                for kc in range(KC):
                    _mmr(nc,
                        psP, c2p_sb[:, si, kc, :], x[kc][:, :],
                        start=(kc == 0), stop=(kc == KC - 1),
                    )
                P_sb = lp.tile([64, BL, 7, 7], f32, tag="P")
                nc.scalar.activation(
                    P_sb, psP, AF.Identity, bias=c2pb_sb[:, si:si + 1], scale=1.0
                )
                n_i = BL * HI[si] * 7
                nc.vector.tensor_copy(
                    preds_cat[:, OFFS[si]:OFFS[si] + n_i],
                    P_sb[:, :, 0:HI[si], :],
                )
            # label logits: preds . target[label]; labels are same-image patches
            prod = lp1.tile([64, ROWS], f32r)
            Tr = T_sb[:, :].rearrange("t (b r c) -> t b r c", r=7, c=7)
            for si in range(3):
                n_i = BL * HI[si] * 7
                nc.vector.tensor_mul(
                    prod[:, OFFS[si]:OFFS[si] + n_i],
                    preds_cat[:, OFFS[si]:OFFS[si] + n_i],
                    Tr[:, :, si + 3:si + 3 + HI[si], :],
                )
            ones_sb = lp1.tile([64, 1], f32r)
            nc.vector.memset(ones_sb[:, :].bitcast(f32), 1.0)
            psL = ps4.tile([1, ROWS], f32, tag="pl")
            _mmr(nc, psL, ones_sb, prod, start=True, stop=True)
            ll_sb = lp1.tile([1, ROWS], f32)
            nc.scalar.copy(ll_sb, psL)
            nc.sync.dma_start(out=out_ll[:], in_=ll_sb)

            # logits against all gathered targets, streamed row-block-wise
            Tfull = lp1.tile([64, cfg.ncores, NP], f32r)
            nc.sync.dma_start(
                out=Tfull,
                in_=tg_out[:, :, :].rearrange("c t p -> t c p"),
            )
            tf = Tfull[:, :, :]
            tflat = bass.AP(tensor=tf.tensor, offset=tf.offset, ap=[tf.ap[0], [1, GC]])
            n_mb = 4 if ROWS % 4 == 0 and ROWS // 4 <= 128 else 1
            MB = ROWS // n_mb
            csz = 448
            n_ch = (GC + csz - 1) // csz
            for m4 in range(n_mb):
                lg = lp.tile([MB, GC], f32, tag="lg")
                for nch in range(n_ch):
                    w = min(csz, GC - nch * csz)
                    ps = ps4.tile([MB, csz], f32, tag="plg")
                    _mmr(nc,
                        ps[:, 0:w],
                        preds_cat[:, m4 * MB:(m4 + 1) * MB],
                        _apv(tflat, [tflat.ap[0], [1, w]], nch * csz),
                        start=True, stop=True,
                    )
                    nc.scalar.copy(lg[:, nch * csz:nch * csz + w], ps[:, 0:w])
                mx = lp.tile([MB, 1], f32, tag="mx")
                nc.vector.reduce_max(out=mx, in_=lg, axis=AX.X)
                nmx = lp.tile([MB, 1], f32, tag="nmx")
                nc.vector.tensor_scalar_mul(nmx, mx, -1.0)
                ex = lp.tile([MB, GC], f32, tag="ex")
                nc.scalar.activation(ex, lg, AF.Exp, bias=nmx, scale=1.0)
                zz = lp.tile([MB, 1], f32, tag="zz")
                nc.vector.reduce_sum(out=zz, in_=ex, axis=AX.X)
                lnz = lp.tile([MB, 1], f32, tag="lnz")
                nc.scalar.activation(lnz, zz, AF.Ln)
                mlz = lp.tile([MB, 1], f32, tag="mlz")
                nc.vector.tensor_add(mlz, mx, lnz)
                nc.sync.dma_start(out=out_mlz[m4 * MB:(m4 + 1) * MB], in_=mlz)

    return nc


# ---------------------------------------------------------------------------


def host_prep(inputs, cfg: Cfg):
    """Per-core input maps. Only sharding, layout transforms of weights, and
    index-arithmetic mask vectors happen here — all FLOPs stay on device."""
    LAT, NBLK = cfg.latent, cfg.nblk
    EMB_SCALE = 0.1
    images = np.ascontiguousarray(inputs["images"], dtype=np.float32)
    rnd = np.asarray(inputs["rnd"]).astype(np.int32)
    B = images.shape[0]

    r0 = (rnd // 4).astype(np.int64)
    c0 = (rnd % 4).astype(np.int64)
    idx = np.arange(64)
    rowm_all = ((idx[None, :] >= r0[:, None]) & (idx[None, :] < r0[:, None] + 60)).astype(np.float32)
    colm_all = ((idx[None, :] >= c0[:, None]) & (idx[None, :] < c0[:, None] + 60)).astype(np.float32)
    # device layouts: rowmx[b, c*8+kh, (pr,pc,oh)] = rowm[p, 8*oh+kh]
    #                 colmx[b, c*8+kh, (pr,pc,x)]  = colm[p, x]
    rm_p = rowm_all.reshape(B, 49, 8, 8)                       # (b, p, oh, kh)
    rowmx_all = np.broadcast_to(
        rm_p.transpose(0, 3, 1, 2)[:, None, :, :, :], (B, 3, 8, 49, 8)
    ).reshape(B, 24, 392).astype(np.float32)
    cm_p = colm_all.reshape(B, 49, 64)
    colmx_all = np.broadcast_to(
        cm_p[:, None, None, :, :], (B, 3, 8, 49, 64)
    ).reshape(B, 24, 3136).astype(np.float32)

    w1 = np.asarray(inputs["enc_w1"], dtype=np.float32)          # (256,3,8,8)
    w1r = np.ascontiguousarray(w1.transpose(1, 2, 3, 0).reshape(24, 8, 256))
    b1p = (np.asarray(inputs["enc_b1"], np.float32) - w1.sum(axis=(1, 2, 3)))
    w2t = np.ascontiguousarray(np.asarray(inputs["enc_w2"], np.float32)[:, :, 0, 0].T)  # (256,LAT)
    b2 = np.asarray(inputs["enc_b2"], np.float32)
    pc1t = np.ascontiguousarray(np.asarray(inputs["pc_w1"], np.float32)[:, :, :, 0, 0].transpose(0, 2, 1))  # (5,LAT,256)
    pc2t = np.ascontiguousarray(np.asarray(inputs["pc_w2"], np.float32)[:, :, :, 0, :].transpose(0, 3, 2, 1))  # (5,3,256in,256out)
    pc3t = np.ascontiguousarray(np.asarray(inputs["pc_w3"], np.float32)[:, :, :, :, 0].transpose(0, 3, 2, 1))  # (5,2,256in,256out)
    pc4t = np.ascontiguousarray(np.asarray(inputs["pc_w4"], np.float32)[:, :, :, 0, 0].transpose(0, 2, 1))  # (5,256,LAT)
    z2tt = np.ascontiguousarray(np.asarray(inputs["z2t_w"], np.float32)[:, :, 0, 0].T)  # (LAT,64)
    c2pt = np.ascontiguousarray(np.asarray(inputs["c2p_w"], np.float32)[:, :, :, 0, 0].transpose(0, 2, 1)) * EMB_SCALE
    c2pb = np.asarray(inputs["c2p_b"], np.float32) * EMB_SCALE

    MT = cfg.mt
    mkb = lambda v, n: np.ascontiguousarray(
        np.asarray(v, np.float32).reshape(NBLK, n, 128).transpose(2, 0, 1)
    )
    import ml_dtypes
    bfl = ml_dtypes.bfloat16
    shared = {
        "w1r": w1r.astype(bfl),
        "b1p": np.ascontiguousarray(b1p.reshape(2, 128).T),
        "w2t": w2t,
        "b2": np.ascontiguousarray(b2.reshape(MT, 128).T),
        "pc1t": pc1t, "pcb1": mkb(inputs["pc_b1"], 2),
        "pc2t": pc2t, "pcb2": mkb(inputs["pc_b2"], 2),
        "pc3t": pc3t, "pcb3": mkb(inputs["pc_b3"], 2),
        "pc4t": pc4t, "pcb4": mkb(inputs["pc_b4"], MT),
        "z2tt": z2tt, "z2tb": np.asarray(inputs["z2t_b"], np.float32),
        "c2pt": c2pt, "c2pb": np.ascontiguousarray(c2pb.T),
    }
    in_maps = []
    for c in range(cfg.ncores):
        bsl = slice(c * cfg.bl, (c + 1) * cfg.bl)
        m = dict(shared)
        m["images"] = np.ascontiguousarray(images[bsl]).astype(bfl)
        m["rowmx"] = np.ascontiguousarray(rowmx_all[bsl]).astype(bfl)
        m["colmx"] = np.ascontiguousarray(colmx_all[bsl]).astype(bfl)
        in_maps.append(m)
    return in_maps


def host_epilogue(results, cfg: Cfg):
    B, BL = cfg.ncores * cfg.bl, cfg.bl
    HI = (4, 3, 2)
    OFFS = (0, BL * 28, BL * 49, BL * 63)
    loss = np.float64(0.0)
    for si in range(3):
        tot = B * HI[si] * 7
        s = np.float64(0.0)
        for r in results:
            sl = slice(OFFS[si], OFFS[si + 1])
            s += (r["out_mlz"][sl].astype(np.float64) - r["out_ll"][sl].astype(np.float64)).sum()
        loss += s / tot
    latmean = np.concatenate([r["out_lm"].T for r in results], axis=0)  # (B, LAT)
    return np.float32(loss), latmean.astype(np.float32)


_CACHE = {}


def kernel(**inputs):
    cfg = Cfg()
    if "nc" not in _CACHE:
        _CACHE["nc"] = build_nc(cfg)
    nc = _CACHE["nc"]
    in_maps = host_prep(inputs, cfg)
    from concourse.bass_utils import run_bass_kernel_spmd

    if not _CACHE.get("split_done"):
        _split_multiwait(nc)
        _CACHE["split_done"] = True
    res = run_bass_kernel_spmd(nc, in_maps, list(range(cfg.ncores)))
    return host_epilogue(res.results, cfg)


# revision 45
# speedup vs baseline: 2.2929x; 1.0016x over previous
"""CPC (contrastive predictive coding) forward pass on 8 Trainium2 NeuronCores.

Data-parallel over the batch: each core processes 8 images (392 patches).
Single SPMD launch; the contrastive targets are exchanged with an on-device
AllGather so each core can normalize its logits over all 3136 negatives.

Encoder conv1 (3->256, k8 s8) runs as 8 PSUM-accumulated matmuls over the
kernel-column index kw with K=(channel, kernel-row)=24 on the partitions and
strided free-dim slices of the patch-row tensor B[(c,kh), (patch,oh,x)] —
this keeps every DMA from HBM at 256B-contiguous runs (no im2col gather).
Border masking is (p+1)*rowmask*colmask - 1 folded into conv1's bias.
"""

import sys
from contextlib import ExitStack
from dataclasses import dataclass

import numpy as np

for _p in ("/opt/trn_rl_repo", "/root/.axon_site/_ro/trn_rl_repo"):
    if _p not in sys.path:
        sys.path.append(_p)

import concourse.bass as bass
import concourse.mybir as mybir
import concourse.tile as tile

f32 = mybir.dt.float32
f32r = mybir.dt.float32r
bf16 = mybir.dt.bfloat16
AF = mybir.ActivationFunctionType
AL = mybir.AluOpType
AX = mybir.AxisListType


@dataclass(frozen=True)
class Cfg:
    ncores: int = 8
    bl: int = 8          # images per core
    latent: int = 2048
    nblk: int = 5        # pixelcnn residual blocks
    use_collective: bool = True
    sim_safe: bool = False   # memset pad partitions so CoreSim sees no uninit reads

    @property
    def mt(self):        # latent 128-tiles
        return self.latent // 128

    @property
    def np_(self):       # patches per core
        return self.bl * 49

    @property
    def rows(self):      # contrastive rows per core (steps 2,3,4 kept rows)
        return self.bl * (4 + 3 + 2) * 7

    @property
    def gcols(self):     # global negatives
        return self.ncores * self.np_


def _apv(base, dims, extra_off=0):
    return bass.AP(tensor=base.tensor, offset=base.offset + extra_off, ap=list(dims))


def _split_multiwait(nc):
    """This env's walrus accepts only one sync-wait per instruction; split
    extras onto single-wait NoOps placed just before."""
    for fn in nc.m.functions:
        for blk in fn.blocks:
            new_insts = []
            for inst in blk.instructions:
                si = inst.sync_info
                if si is not None and len(si.on_wait) > 1:
                    waits = list(si.on_wait)
                    for j, w in enumerate(waits[:-1]):
                        new_insts.append(
                            mybir.InstNoOp(
                                name=f"{inst.name}-wsplit{j}",
                                sync_info=mybir.SyncInfo(on_wait=[w], on_update=[]),
                                bass_nofuse=True,
                                engine=inst.engine,
                            )
                        )
                    inst.sync_info = mybir.SyncInfo(
                        on_wait=[waits[-1]], on_update=list(si.on_update)
                    )
                new_insts.append(inst)
            blk.instructions = new_insts


def _mmr(nc, out, lhsT, rhs, **kw):
    """Matmul on float32r-typed operands (full-rate fp32). The f32r path
    streams element pairs, so odd free sizes fall back to plain fp32."""
    if rhs.free_size() % 2:
        nc.tensor.matmul(out, lhsT.bitcast(f32), rhs.bitcast(f32), **kw)
    else:
        nc.tensor.matmul(out, lhsT, rhs, **kw)


def build_nc(cfg: Cfg):
    BL, MT, NBLK, NP = cfg.bl, cfg.mt, cfg.nblk, cfg.np_
    LAT = cfg.latent
    KC = MT  # 128-chunks of the latent contraction
    ROWS = cfg.rows
    GC = cfg.gcols
    HI = (4, 3, 2)
    OFFS = (0, BL * 28, BL * 49)  # row offsets of the 3 steps in preds_cat

    nc = bass.Bass()
    dp = nc.declare_dram_parameter
    images = dp("images", [BL, 3, 256, 256], bf16, isOutput=False)
    rowmx = dp("rowmx", [BL, 24, 392], bf16, isOutput=False)   # (b,(c,kh),(pr,pc,oh))
    colmx = dp("colmx", [BL, 24, 3136], bf16, isOutput=False)  # (b,(c,kh),(pr,pc,x))
    w1r = dp("w1r", [24, 8, 256], bf16, isOutput=False)      # (c*8+kh, kw, co)
    b1p = dp("b1p", [128, 2], f32, isOutput=False)           # b1 - sum(W1)
    w2t = dp("w2t", [256, LAT], f32r, isOutput=False)
    b2 = dp("b2", [128, MT], f32, isOutput=False)
    pc1t = dp("pc1t", [NBLK, LAT, 256], f32r, isOutput=False)
    pcb1 = dp("pcb1", [128, NBLK, 2], f32, isOutput=False)
    pc2t = dp("pc2t", [NBLK, 3, 256, 256], f32r, isOutput=False)
    pcb2 = dp("pcb2", [128, NBLK, 2], f32, isOutput=False)
    pc3t = dp("pc3t", [NBLK, 2, 256, 256], f32r, isOutput=False)
    pcb3 = dp("pcb3", [128, NBLK, 2], f32, isOutput=False)
    pc4t = dp("pc4t", [NBLK, 256, LAT], f32r, isOutput=False)
    pcb4 = dp("pcb4", [128, NBLK, MT], f32, isOutput=False)
    z2tt = dp("z2tt", [LAT, 64], f32r, isOutput=False)
    z2tb = dp("z2tb", [64], f32, isOutput=False)
    c2pt = dp("c2pt", [3, LAT, 64], f32r, isOutput=False)     # pre-scaled by 0.1
    c2pb = dp("c2pb", [64, 3], f32, isOutput=False)
    out_mlz = dp("out_mlz", [ROWS], f32, isOutput=True)      # max + logsumexp per row
    out_ll = dp("out_ll", [ROWS], f32, isOutput=True)        # label logit per row
    out_lm = dp("out_lm", [LAT, BL], f32, isOutput=True)     # latents spatial mean

    tg_in = nc.dram_tensor("tg_in", [64, NP], f32r)
    if cfg.use_collective:
        tg_out = nc.dram_tensor("tg_out", [cfg.ncores, 64, NP], f32r, addr_space="Shared")
    else:
        tg_out = nc.dram_tensor("tg_out", [cfg.ncores, 64, NP], f32r)

    QG = min(4, BL)           # images per partition-group tile
    QUADS = [list(range(q, min(q + QG, BL))) for q in range(0, BL, QG)]
    HALVES = ((0, 4), (4, 3)) if True else None  # pr-rows split 4+3

    with tile.TileContext(nc) as tc, ExitStack() as top:
        pers = top.enter_context(tc.tile_pool(name="pers", bufs=1))
        pst = top.enter_context(tc.tile_pool(name="pst", bufs=1, space="PSUM"))

        lat = [pers.tile([128, NP], f32r, tag=f"lat{m}", name=f"lat{m}") for m in range(MT)]
        latb = [pers.tile([128, NP], bf16, tag=f"latb{m}", name=f"latb{m}") for m in range(MT)]

        # ---------------- encoder ----------------
        with ExitStack() as enc:
            ew = enc.enter_context(tc.tile_pool(name="encw", bufs=1))
            ep = enc.enter_context(tc.tile_pool(name="enc", bufs=2))
            hp = enc.enter_context(tc.tile_pool(name="hp", bufs=2))
            hcp = enc.enter_context(tc.tile_pool(name="hcp", bufs=6))
            ps1 = enc.enter_context(tc.tile_pool(name="ps1", bufs=3, space="PSUM"))
            ps2 = enc.enter_context(tc.tile_pool(name="ps2", bufs=2, space="PSUM"))

            w1_sb = ew.tile([128, 8, 256], bf16)
            for g in range(QG):
                nc.sync.dma_start(
                    out=w1_sb[32 * g:32 * g + 24], in_=w1r[:, :, :]
                )
            b1d = ew.tile([128, 2], f32)
            nc.sync.dma_start(out=b1d, in_=b1p[:, :])
            nc.vector.tensor_scalar_mul(b1d, b1d, 1.0 / 64.0)
            w2a = ew.tile([128, LAT], f32r)
            w2b = ew.tile([128, LAT], f32r)
            nc.sync.dma_start(out=w2a, in_=w2t[0:128, :])
            nc.sync.dma_start(out=w2b, in_=w2t[128:256, :])
            b2d = ew.tile([128, MT], f32)
            nc.sync.dma_start(out=b2d, in_=b2[:, :])
            nc.vector.tensor_scalar_mul(b2d, b2d, 1.0 / 64.0)

            for quad in QUADS:
                for pr0, npr in HALVES:
                    # Brow: partition group 32g holds image quad[g], rows
                    # (c*8+kh); free = (pr, oh, col) raw image rows, deduped.
                    Brow = ep.tile([128, npr, 8, 256], bf16, tag="Brow")
                    # B32: per-patch expanded+masked view, free (pr,pc,oh,x)
                    B32 = ep.tile([128, npr, 8, 7, 64], bf16, tag="B32")
                    RM32 = ep.tile([128, npr, 7, 8], bf16, tag="RM32")
                    CM32 = ep.tile([128, npr, 7, 64], bf16, tag="CM32")
                    if cfg.sim_safe:
                        nc.gpsimd.memset(Brow, 0.0)
                        nc.gpsimd.memset(RM32, 0.0)
                        nc.gpsimd.memset(CM32, 0.0)
                    for g, img in enumerate(quad):
                        for c in range(3):
                            for prl in range(npr):
                                ioff = (img * 3 + c) * 65536 + (pr0 + prl) * 8192
                                deng = nc.sync if (c + prl) % 2 == 0 else nc.scalar
                                deng.dma_start(
                                    out=Brow[32 * g + 8 * c:32 * g + 8 * c + 8, prl],
                                    in_=_apv(
                                        images[:, :, :, :],
                                        [[256, 8], [2048, 8], [1, 256]],
                                        ioff,
                                    ),
                                )
                        moff = (img * 24) * 392 + pr0 * 56
                        nc.scalar.dma_start(
                            out=RM32[32 * g:32 * g + 24],
                            in_=_apv(rowmx[:, :, :], [[392, 24], [1, npr * 56]], moff),
                        )
                        moff = (img * 24) * 3136 + pr0 * 448
                        nc.scalar.dma_start(
                            out=CM32[32 * g:32 * g + 24],
                            in_=_apv(colmx[:, :, :], [[3136, 24], [1, npr * 448]], moff),
                        )
                    # masked = (B+1)*RM*CM; pass 1 also expands rows to
                    # overlapping patches via an overlapping-stride read view
                    # (one 3D op per (pr, oh) - walrus limits stt to 3D).
                    # B32 free order is (pr, oh, pc, x) so pass 2's column
                    # mask broadcasts over oh in ONE 3D op per pr.
                    brf = Brow[:, :, :, :]
                    rmf = RM32[:, :, :, :]
                    cmf = CM32[:, :, :, :]
                    b32f = B32[:, :, :, :, :]
                    for prl in range(npr):
                        for oh in range(8):
                            nc.vector.scalar_tensor_tensor(
                                out=B32[:, prl, oh, :, :], scalar=1.0,
                                in0=_apv(brf, [brf.ap[0], [32, 7], [1, 64]],
                                         prl * 2048 + oh * 256),
                                in1=_apv(rmf, [rmf.ap[0], [8, 7], [0, 64]],
                                         prl * 56 + oh),
                                op0=AL.add, op1=AL.mult,
                            )
                        b32sl = _apv(b32f, [b32f.ap[0], [448, 8], [1, 448]],
                                     prl * 3584)
                        nc.vector.scalar_tensor_tensor(
                            out=b32sl, in0=b32sl, scalar=0.0,
                            in1=_apv(cmf, [cmf.ap[0], [0, 8], [1, 448]],
                                     prl * 448),
                            op0=AL.add, op1=AL.mult,
                        )
                    pairs = [(0, 1), (2, 3)] if npr == 4 else [(0, 1), (2,)]
                    for g, img in enumerate(quad):
                        bsl = B32[32 * g:32 * g + 24]
                        for pair in pairs:
                            # conv1 for the pr-pair; h pre-scaled by 1/64 so
                            # the conv2 epilogue is a plain bias+relu
                            h_pp = {}
                            for j, prl in enumerate(pair):
                                for cot in range(2):
                                    ps = ps1.tile([128, 448], f32, tag="c1")
                                    for kw in range(8):
                                        rhs = _apv(
                                            bsl,
                                            [bsl.ap[0], [64, 7], [448, 8], [8, 8]],
                                            prl * 3584 + kw,
                                        )
                                        nc.tensor.matmul(
                                            ps,
                                            w1_sb[32 * g:32 * g + 24, kw,
                                                  cot * 128:(cot + 1) * 128],
                                            rhs, start=(kw == 0), stop=(kw == 7),
                                            tile_position=(32 * g, 0),
                                        )
                                    h = hp.tile([128, 448], f32r, tag=f"h{j}{cot}")
                                    nc.scalar.activation(
                                        h, ps, AF.Relu, bias=b1d[:, cot:cot + 1],
                                        scale=1.0 / 64.0,
                                    )
                                    h_pp[(j, cot)] = h
                            base = img * 49 + (pr0 + pair[0]) * 7
                            w = 7 * len(pair)
                            for m in range(MT):
                                ps = ps2.tile([128, 2, 512], f32, tag="c2")
                                for j, prl in enumerate(pair):
                                    _mmr(nc,
                                        ps[:, j, 0:448],
                                        w2a[:, m * 128:(m + 1) * 128], h_pp[(j, 0)],
                                        start=True, stop=False,
                                    )
                                    _mmr(nc,
                                        ps[:, j, 0:448],
                                        w2b[:, m * 128:(m + 1) * 128], h_pp[(j, 1)],
                                        start=False, stop=True,
                                    )
                                psf = ps[:, :, :]
                                psv = _apv(psf, [psf.ap[0], [512, len(pair)], [1, 448]])
                                hc = hcp.tile([128, 2, 7, 64], bf16, tag="hc")
                                hcf = hc[:, :, :, :]
                                hcv = _apv(hcf, [hcf.ap[0], [448, len(pair)], [1, 448]])
                                eng = nc.scalar
                                if eng is nc.scalar:
                                    nc.scalar.activation(
                                        hcv, psv, AF.Relu, bias=b2d[:, m:m + 1],
                                        scale=1.0,
                                    )
                                else:
                                    eng.tensor_scalar(
                                        hcv, psv, b2d[:, m:m + 1], 0.0,
                                        op0=AL.add, op1=AL.max,
                                    )
                                with nc.allow_low_precision(
                                    reason="64-elem pool; bf16 out is plenty"
                                ):
                                    nc.vector.reduce_sum(
                                        out=latb[m][:, base:base + w],
                                    in_=_apv(hcf, [hcf.ap[0], [64, w], [1, 64]]),
                                    axis=AX.X,
                                )

        for m in range(MT):
            nc.vector.tensor_copy(lat[m][:, :], latb[m][:, :])

        # ---------------- latents mean + targets (+ gather) ----------------
        z2t_sb = pers.tile([128, KC, 64], f32r)
        nc.sync.dma_start(
            out=z2t_sb, in_=_apv(z2tt[:, :], [[64, 128], [8192, KC], [1, 64]])
        )
        z2tb_sb = pers.tile([64, 1], f32)
        nc.sync.dma_start(out=z2tb_sb, in_=_apv(z2tb[:], [[1, 64], [0, 1]]))
        for m in range(MT):
            lm = pers.tile([128, BL], f32, tag="lm", name="lm", bufs=2)
            nc.vector.reduce_sum(
                out=lm,
                in_=lat[m][:, :].rearrange("p (b s) -> p b s", s=49),
                axis=AX.X,
            )
            nc.vector.tensor_scalar_mul(lm, lm, 1.0 / 49.0)
            nc.sync.dma_start(out=out_lm[m * 128:(m + 1) * 128, :], in_=lm)

        psT = pst.tile([64, NP], f32, tag="pt")
        for kc in range(KC):
            _mmr(nc,
                psT, z2t_sb[:, kc, :], lat[kc][:, :],
                start=(kc == 0), stop=(kc == KC - 1),
            )
        T_sb = pers.tile([64, NP], f32r)
        nc.scalar.activation(T_sb, psT, AF.Identity, bias=z2tb_sb[:, 0:1], scale=1.0)
        nc.sync.dma_start(out=tg_in[:, :], in_=T_sb)
        if cfg.use_collective:
            nc.gpsimd.collective_compute(
                "AllGather",
                AL.bypass,
                replica_groups=[list(range(cfg.ncores))],
                ins=[tg_in[:, :]],
                outs=[tg_out[:, :, :]],
            )
        else:
            nc.gpsimd.dma_start(out=tg_out[0], in_=tg_in[:, :])

        # ---------------- pixelcnn ----------------
        x = list(lat)
        with ExitStack() as pcs:
            pw = pcs.enter_context(tc.tile_pool(name="pcw", bufs=2))
            yp = pcs.enter_context(tc.tile_pool(name="yp", bufs=2))
            ps3 = pcs.enter_context(tc.tile_pool(name="ps3", bufs=3, space="PSUM"))

            pb1_sb = pers.tile([128, NBLK, 2], f32)
            nc.sync.dma_start(
                out=pb1_sb, in_=pcb1[:, :, :]
            )
            pb2_sb = pers.tile([128, NBLK, 2], f32)
            nc.sync.dma_start(
                out=pb2_sb, in_=pcb2[:, :, :]
            )
            pb3_sb = pers.tile([128, NBLK, 2], f32)
            nc.sync.dma_start(
                out=pb3_sb, in_=pcb3[:, :, :]
            )
            pb4_sb = pers.tile([128, NBLK, MT], f32)
            nc.sync.dma_start(
                out=pb4_sb, in_=pcb4[:, :, :]
            )

            for k in range(NBLK):
                pc1_sb = pw.tile([128, KC, 256], f32r, tag="pc1")
                nc.sync.dma_start(
                    out=pc1_sb,
                    in_=_apv(
                        pc1t[:, :, :], [[256, 128], [32768, KC], [1, 256]],
                        k * LAT * 256,
                    ),
                )
                pc2_sb = pw.tile([128, 3, 2, 256], f32r, tag="pc2")
                for dx in range(3):
                    nc.sync.dma_start(
                        out=pc2_sb[:, dx],
                        in_=_apv(
                            pc2t[:, :, :, :],
                            [[256, 128], [32768, 2], [1, 256]],
                            (k * 3 + dx) * 65536,
                        ),
                    )
                pc3_sb = pw.tile([128, 2, 2, 256], f32r, tag="pc3")
                for dy in range(2):
                    nc.sync.dma_start(
                        out=pc3_sb[:, dy],
                        in_=_apv(
                            pc3t[:, :, :, :],
                            [[256, 128], [32768, 2], [1, 256]],
                            (k * 2 + dy) * 65536,
                        ),
                    )
                pc4_sb = pw.tile([128, 2, LAT], f32r, tag="pc4")
                nc.sync.dma_start(
                    out=pc4_sb,
                    in_=_apv(
                        pc4t[:, :, :], [[LAT, 128], [128 * LAT, 2], [1, LAT]],
                        k * 256 * LAT,
                    ),
                )

                # y1 = relu(1x1 conv LAT->256), written into col-padded buffer
                y1p = []
                for m2 in range(2):
                    ps = ps3.tile([128, NP], f32, tag="py")
                    for kc in range(KC):
                        _mmr(nc,
                            ps, pc1_sb[:, kc, m2 * 128:(m2 + 1) * 128], x[kc][:, :],
                            start=(kc == 0), stop=(kc == KC - 1),
                        )
                    t = yp.tile([128, BL, 7, 9], f32r, tag=f"y1p{m2}")
                    nc.vector.memset(t[:, :, :, :].bitcast(f32), 0.0)
                    nc.scalar.activation(
                        t[:, :, :, 1:8], ps, AF.Relu,
                        bias=pb1_sb[:, k, m2:m2 + 1], scale=1.0,
                    )
                    y1p.append(t)
                # y2 = relu(1x3 conv along columns), row-padded buffer
                y2p = []
                for m2 in range(2):
                    ps = ps3.tile([128, NP], f32, tag="py")
                    first = True
                    for dx in range(3):
                        for kc in range(2):
                            nc.tensor.matmul(
                                ps,
                                pc2_sb[:, dx, kc, m2 * 128:(m2 + 1) * 128].bitcast(f32),
                                y1p[kc][:, :, :, dx:dx + 7].bitcast(f32),
                                start=first, stop=(dx == 2 and kc == 1),
                            )
                            first = False
                    t = yp.tile([128, BL, 8, 7], f32r, tag=f"y2p{m2}")
                    nc.vector.memset(t[:, :, :, :].bitcast(f32), 0.0)
                    nc.scalar.activation(
                        t[:, :, 1:8, :], ps, AF.Relu,
                        bias=pb2_sb[:, k, m2:m2 + 1], scale=1.0,
                    )
                    y2p.append(t)
                # y3 = relu(2x1 conv along rows, top pad)
                y3 = []
                for m2 in range(2):
                    ps = ps3.tile([128, NP], f32, tag="py")
                    first = True
                    for dy in range(2):
                        for kc in range(2):
                            nc.tensor.matmul(
                                ps,
                                pc3_sb[:, dy, kc, m2 * 128:(m2 + 1) * 128].bitcast(f32),
                                y2p[kc][:, :, dy:dy + 7, :].bitcast(f32),
                                start=first, stop=(dy == 1 and kc == 1),
                            )
                            first = False
                    t = yp.tile([128, NP], f32r, tag=f"y3{m2}")
                    nc.scalar.activation(
                        t, ps, AF.Relu, bias=pb3_sb[:, k, m2:m2 + 1], scale=1.0
                    )
                    y3.append(t)
                # y4 = 1x1 conv 256->LAT; x = relu(y4 + b4 + x)
                for m in range(MT):
                    ps = ps3.tile([128, NP], f32, tag="py4")
                    for kc in range(2):
                        _mmr(nc,
                            ps, pc4_sb[:, kc, m * 128:(m + 1) * 128], y3[kc][:, :],
                            start=(kc == 0), stop=(kc == 1),
                        )
                    t = yp.tile([128, NP], f32, tag="resid")
                    nc.vector.scalar_tensor_tensor(
                        out=t, in0=ps, scalar=pb4_sb[:, k, m:m + 1], in1=x[m][:, :],
                        op0=AL.add, op1=AL.add,
                    )
                    nc.scalar.activation(x[m][:, :], t, AF.Relu)

        # ---------------- preds + logits + loss pieces ----------------
        with ExitStack() as lgs:
            lp = lgs.enter_context(tc.tile_pool(name="lp", bufs=2))
            lp1 = lgs.enter_context(tc.tile_pool(name="lp1", bufs=1))
            ps4 = lgs.enter_context(tc.tile_pool(name="ps4", bufs=2, space="PSUM"))

            c2p_sb = lp1.tile([128, 3, KC, 64], f32r)
            for si in range(3):
                nc.sync.dma_start(
                    out=c2p_sb[:, si],
                    in_=_apv(c2pt[:, :, :], [[64, 128], [8192, KC], [1, 64]],
                             si * LAT * 64),
                )
            c2pb_sb = lp1.tile([64, 3], f32)
            nc.sync.dma_start(out=c2pb_sb, in_=c2pb[:, :])

            preds_cat = lp1.tile([64, ROWS], f32r)
            for si in range(3):
                psP = ps4.tile([64, NP], f32, tag="pp")
                for kc in range(KC):
                    _mmr(nc,
                        psP, c2p_sb[:, si, kc, :], x[kc][:, :],
                        start=(kc == 0), stop=(kc == KC - 1),
                    )
                P_sb = lp.tile([64, BL, 7, 7], f32, tag="P")
                nc.scalar.activation(
                    P_sb, psP, AF.Identity, bias=c2pb_sb[:, si:si + 1], scale=1.0
                )
                n_i = BL * HI[si] * 7
                nc.vector.tensor_copy(
                    preds_cat[:, OFFS[si]:OFFS[si] + n_i],
                    P_sb[:, :, 0:HI[si], :],
                )
            # label logits: preds . target[label]; labels are same-image patches
            prod = lp1.tile([64, ROWS], f32r)
            Tr = T_sb[:, :].rearrange("t (b r c) -> t b r c", r=7, c=7)
            for si in range(3):
                n_i = BL * HI[si] * 7
                nc.vector.tensor_mul(
                    prod[:, OFFS[si]:OFFS[si] + n_i],
                    preds_cat[:, OFFS[si]:OFFS[si] + n_i],
                    Tr[:, :, si + 3:si + 3 + HI[si], :],
                )
            ones_sb = lp1.tile([64, 1], f32r)
            nc.vector.memset(ones_sb[:, :].bitcast(f32), 1.0)
            psL = ps4.tile([1, ROWS], f32, tag="pl")
            _mmr(nc, psL, ones_sb, prod, start=True, stop=True)
            ll_sb = lp1.tile([1, ROWS], f32)
            nc.scalar.copy(ll_sb, psL)
            nc.sync.dma_start(out=out_ll[:], in_=ll_sb)

            # logits against all gathered targets, streamed row-block-wise
            Tfull = lp1.tile([64, cfg.ncores, NP], f32r)
            nc.sync.dma_start(
                out=Tfull,
                in_=tg_out[:, :, :].rearrange("c t p -> t c p"),
            )
            tf = Tfull[:, :, :]
            tflat = bass.AP(tensor=tf.tensor, offset=tf.offset, ap=[tf.ap[0], [1, GC]])
            n_mb = 4 if ROWS % 4 == 0 and ROWS // 4 <= 128 else 1
            MB = ROWS // n_mb
            csz = 448
            n_ch = (GC + csz - 1) // csz
            for m4 in range(n_mb):
                lg = lp.tile([MB, GC], f32, tag="lg")
                for nch in range(n_ch):
                    w = min(csz, GC - nch * csz)
                    ps = ps4.tile([MB, csz], f32, tag="plg")
                    _mmr(nc,
                        ps[:, 0:w],
                        preds_cat[:, m4 * MB:(m4 + 1) * MB],
                        _apv(tflat, [tflat.ap[0], [1, w]], nch * csz),
                        start=True, stop=True,
                    )
                    nc.scalar.copy(lg[:, nch * csz:nch * csz + w], ps[:, 0:w])
                mx = lp.tile([MB, 1], f32, tag="mx")
                nc.vector.reduce_max(out=mx, in_=lg, axis=AX.X)
                nmx = lp.tile([MB, 1], f32, tag="nmx")
                nc.vector.tensor_scalar_mul(nmx, mx, -1.0)
                ex = lp.tile([MB, GC], f32, tag="ex")
                nc.scalar.activation(ex, lg, AF.Exp, bias=nmx, scale=1.0)
                zz = lp.tile([MB, 1], f32, tag="zz")
                nc.vector.reduce_sum(out=zz, in_=ex, axis=AX.X)
                lnz = lp.tile([MB, 1], f32, tag="lnz")
                nc.scalar.activation(lnz, zz, AF.Ln)
                mlz = lp.tile([MB, 1], f32, tag="mlz")
                nc.vector.tensor_add(mlz, mx, lnz)
                nc.sync.dma_start(out=out_mlz[m4 * MB:(m4 + 1) * MB], in_=mlz)

    return nc


# ---------------------------------------------------------------------------


def host_prep(inputs, cfg: Cfg):
    """Per-core input maps. Only sharding, layout transforms of weights, and
    index-arithmetic mask vectors happen here — all FLOPs stay on device."""
    LAT, NBLK = cfg.latent, cfg.nblk
    EMB_SCALE = 0.1
    images = np.ascontiguousarray(inputs["images"], dtype=np.float32)
    rnd = np.asarray(inputs["rnd"]).astype(np.int32)
    B = images.shape[0]

    r0 = (rnd // 4).astype(np.int64)
    c0 = (rnd % 4).astype(np.int64)
    idx = np.arange(64)
    rowm_all = ((idx[None, :] >= r0[:, None]) & (idx[None, :] < r0[:, None] + 60)).astype(np.float32)
    colm_all = ((idx[None, :] >= c0[:, None]) & (idx[None, :] < c0[:, None] + 60)).astype(np.float32)
    # device layouts: rowmx[b, c*8+kh, (pr,pc,oh)] = rowm[p, 8*oh+kh]
    #                 colmx[b, c*8+kh, (pr,pc,x)]  = colm[p, x]
    rm_p = rowm_all.reshape(B, 49, 8, 8)                       # (b, p, oh, kh)
    rowmx_all = np.broadcast_to(
        rm_p.transpose(0, 3, 1, 2)[:, None, :, :, :], (B, 3, 8, 49, 8)
    ).reshape(B, 24, 392).astype(np.float32)
    cm_p = colm_all.reshape(B, 49, 64)
    colmx_all = np.broadcast_to(
        cm_p[:, None, None, :, :], (B, 3, 8, 49, 64)
    ).reshape(B, 24, 3136).astype(np.float32)

    w1 = np.asarray(inputs["enc_w1"], dtype=np.float32)          # (256,3,8,8)
    w1r = np.ascontiguousarray(w1.transpose(1, 2, 3, 0).reshape(24, 8, 256))
    b1p = (np.asarray(inputs["enc_b1"], np.float32) - w1.sum(axis=(1, 2, 3)))
    w2t = np.ascontiguousarray(np.asarray(inputs["enc_w2"], np.float32)[:, :, 0, 0].T)  # (256,LAT)
    b2 = np.asarray(inputs["enc_b2"], np.float32)
    pc1t = np.ascontiguousarray(np.asarray(inputs["pc_w1"], np.float32)[:, :, :, 0, 0].transpose(0, 2, 1))  # (5,LAT,256)
    pc2t = np.ascontiguousarray(np.asarray(inputs["pc_w2"], np.float32)[:, :, :, 0, :].transpose(0, 3, 2, 1))  # (5,3,256in,256out)
    pc3t = np.ascontiguousarray(np.asarray(inputs["pc_w3"], np.float32)[:, :, :, :, 0].transpose(0, 3, 2, 1))  # (5,2,256in,256out)
    pc4t = np.ascontiguousarray(np.asarray(inputs["pc_w4"], np.float32)[:, :, :, 0, 0].transpose(0, 2, 1))  # (5,256,LAT)
    z2tt = np.ascontiguousarray(np.asarray(inputs["z2t_w"], np.float32)[:, :, 0, 0].T)  # (LAT,64)
    c2pt = np.ascontiguousarray(np.asarray(inputs["c2p_w"], np.float32)[:, :, :, 0, 0].transpose(0, 2, 1)) * EMB_SCALE
    c2pb = np.asarray(inputs["c2p_b"], np.float32) * EMB_SCALE

    MT = cfg.mt
    mkb = lambda v, n: np.ascontiguousarray(
        np.asarray(v, np.float32).reshape(NBLK, n, 128).transpose(2, 0, 1)
    )
    import ml_dtypes
    bfl = ml_dtypes.bfloat16
    shared = {
        "w1r": w1r.astype(bfl),
        "b1p": np.ascontiguousarray(b1p.reshape(2, 128).T),
        "w2t": w2t,
        "b2": np.ascontiguousarray(b2.reshape(MT, 128).T),
        "pc1t": pc1t, "pcb1": mkb(inputs["pc_b1"], 2),
        "pc2t": pc2t, "pcb2": mkb(inputs["pc_b2"], 2),
        "pc3t": pc3t, "pcb3": mkb(inputs["pc_b3"], 2),
        "pc4t": pc4t, "pcb4": mkb(inputs["pc_b4"], MT),
        "z2tt": z2tt, "z2tb": np.asarray(inputs["z2t_b"], np.float32),
        "c2pt": c2pt, "c2pb": np.ascontiguousarray(c2pb.T),
    }
    in_maps = []
    for c in range(cfg.ncores):
        bsl = slice(c * cfg.bl, (c + 1) * cfg.bl)
        m = dict(shared)
        m["images"] = np.ascontiguousarray(images[bsl]).astype(bfl)
        m["rowmx"] = np.ascontiguousarray(rowmx_all[bsl]).astype(bfl)
        m["colmx"] = np.ascontiguousarray(colmx_all[bsl]).astype(bfl)
        in_maps.append(m)
    return in_maps


def host_epilogue(results, cfg: Cfg):
    B, BL = cfg.ncores * cfg.bl, cfg.bl
    HI = (4, 3, 2)
    OFFS = (0, BL * 28, BL * 49, BL * 63)
    loss = np.float64(0.0)
    for si in range(3):
        tot = B * HI[si] * 7
        s = np.float64(0.0)
        for r in results:
            sl = slice(OFFS[si], OFFS[si + 1])
            s += (r["out_mlz"][sl].astype(np.float64) - r["out_ll"][sl].astype(np.float64)).sum()
        loss += s / tot
    latmean = np.concatenate([r["out_lm"].T for r in results], axis=0)  # (B, LAT)
    return np.float32(loss), latmean.astype(np.float32)


_CACHE = {}


def kernel(**inputs):
    cfg = Cfg()
    if "nc" not in _CACHE:
        _CACHE["nc"] = build_nc(cfg)
    nc = _CACHE["nc"]
    in_maps = host_prep(inputs, cfg)
    from concourse.bass_utils import run_bass_kernel_spmd

    if not _CACHE.get("split_done"):
        _split_multiwait(nc)
        _CACHE["split_done"] = True
    res = run_bass_kernel_spmd(nc, in_maps, list(range(cfg.ncores)))
    return host_epilogue(res.results, cfg)


# revision 47
# speedup vs baseline: 2.3438x; 1.0222x over previous
"""CPC (contrastive predictive coding) forward pass on 8 Trainium2 NeuronCores.

Data-parallel over the batch: each core processes 8 images (392 patches).
Single SPMD launch; the contrastive targets are exchanged with an on-device
AllGather so each core can normalize its logits over all 3136 negatives.

Encoder conv1 (3->256, k8 s8) runs as 8 PSUM-accumulated matmuls over the
kernel-column index kw with K=(channel, kernel-row)=24 on the partitions and
strided free-dim slices of the patch-row tensor B[(c,kh), (patch,oh,x)] —
this keeps every DMA from HBM at 256B-contiguous runs (no im2col gather).
Border masking is (p+1)*rowmask*colmask - 1 folded into conv1's bias.
"""

import sys
from contextlib import ExitStack
from dataclasses import dataclass

import numpy as np

for _p in ("/opt/trn_rl_repo", "/root/.axon_site/_ro/trn_rl_repo"):
    if _p not in sys.path:
        sys.path.append(_p)

import concourse.bass as bass
import concourse.mybir as mybir
import concourse.tile as tile

f32 = mybir.dt.float32
f32r = mybir.dt.float32r
bf16 = mybir.dt.bfloat16
AF = mybir.ActivationFunctionType
AL = mybir.AluOpType
AX = mybir.AxisListType


@dataclass(frozen=True)
class Cfg:
    ncores: int = 8
    bl: int = 8          # images per core
    latent: int = 2048
    nblk: int = 5        # pixelcnn residual blocks
    use_collective: bool = True
    sim_safe: bool = False   # memset pad partitions so CoreSim sees no uninit reads

    @property
    def mt(self):        # latent 128-tiles
        return self.latent // 128

    @property
    def np_(self):       # patches per core
        return self.bl * 49

    @property
    def rows(self):      # contrastive rows per core (steps 2,3,4 kept rows)
        return self.bl * (4 + 3 + 2) * 7

    @property
    def gcols(self):     # global negatives
        return self.ncores * self.np_


def _apv(base, dims, extra_off=0):
    return bass.AP(tensor=base.tensor, offset=base.offset + extra_off, ap=list(dims))


def _split_multiwait(nc):
    """This env's walrus accepts only one sync-wait per instruction; split
    extras onto single-wait NoOps placed just before."""
    for fn in nc.m.functions:
        for blk in fn.blocks:
            new_insts = []
            for inst in blk.instructions:
                si = inst.sync_info
                if si is not None and len(si.on_wait) > 1:
                    waits = list(si.on_wait)
                    for j, w in enumerate(waits[:-1]):
                        new_insts.append(
                            mybir.InstNoOp(
                                name=f"{inst.name}-wsplit{j}",
                                sync_info=mybir.SyncInfo(on_wait=[w], on_update=[]),
                                bass_nofuse=True,
                                engine=inst.engine,
                            )
                        )
                    inst.sync_info = mybir.SyncInfo(
                        on_wait=[waits[-1]], on_update=list(si.on_update)
                    )
                new_insts.append(inst)
            blk.instructions = new_insts


def _mmr(nc, out, lhsT, rhs, **kw):
    """Matmul on float32r-typed operands (full-rate fp32). The f32r path
    streams element pairs, so odd free sizes fall back to plain fp32."""
    if rhs.free_size() % 2:
        nc.tensor.matmul(out, lhsT.bitcast(f32), rhs.bitcast(f32), **kw)
    else:
        nc.tensor.matmul(out, lhsT, rhs, **kw)


def build_nc(cfg: Cfg):
    BL, MT, NBLK, NP = cfg.bl, cfg.mt, cfg.nblk, cfg.np_
    LAT = cfg.latent
    KC = MT  # 128-chunks of the latent contraction
    ROWS = cfg.rows
    GC = cfg.gcols
    HI = (4, 3, 2)
    OFFS = (0, BL * 28, BL * 49)  # row offsets of the 3 steps in preds_cat

    nc = bass.Bass()
    dp = nc.declare_dram_parameter
    images = dp("images", [BL, 3, 8, 32, 256], bf16, isOutput=False)  # rows regrouped (kh, y//8)
    rowmx = dp("rowmx", [BL, 24, 392], bf16, isOutput=False)   # (b,(c,kh),(pr,pc,oh))
    colmx = dp("colmx", [BL, 24, 3136], bf16, isOutput=False)  # (b,(c,kh),(pr,pc,x))
    w1r = dp("w1r", [24, 8, 256], bf16, isOutput=False)      # (c*8+kh, kw, co)
    b1p = dp("b1p", [128, 2], f32, isOutput=False)           # b1 - sum(W1)
    w2t = dp("w2t", [256, LAT], f32r, isOutput=False)
    b2 = dp("b2", [128, MT], f32, isOutput=False)
    pc1t = dp("pc1t", [NBLK, LAT, 256], f32r, isOutput=False)
    pcb1 = dp("pcb1", [128, NBLK, 2], f32, isOutput=False)
    pc2t = dp("pc2t", [NBLK, 3, 256, 256], f32r, isOutput=False)
    pcb2 = dp("pcb2", [128, NBLK, 2], f32, isOutput=False)
    pc3t = dp("pc3t", [NBLK, 2, 256, 256], f32r, isOutput=False)
    pcb3 = dp("pcb3", [128, NBLK, 2], f32, isOutput=False)
    pc4t = dp("pc4t", [NBLK, 256, LAT], f32r, isOutput=False)
    pcb4 = dp("pcb4", [128, NBLK, MT], f32, isOutput=False)
    z2tt = dp("z2tt", [LAT, 64], f32r, isOutput=False)
    z2tb = dp("z2tb", [64], f32, isOutput=False)
    c2pt = dp("c2pt", [3, LAT, 64], f32r, isOutput=False)     # pre-scaled by 0.1
    c2pb = dp("c2pb", [64, 3], f32, isOutput=False)
    out_mlz = dp("out_mlz", [ROWS], f32, isOutput=True)      # max + logsumexp per row
    out_ll = dp("out_ll", [ROWS], f32, isOutput=True)        # label logit per row
    out_lm = dp("out_lm", [LAT, BL], f32, isOutput=True)     # latents spatial mean

    tg_in = nc.dram_tensor("tg_in", [64, NP], f32r)
    if cfg.use_collective:
        tg_out = nc.dram_tensor("tg_out", [cfg.ncores, 64, NP], f32r, addr_space="Shared")
    else:
        tg_out = nc.dram_tensor("tg_out", [cfg.ncores, 64, NP], f32r)

    QG = min(4, BL)           # images per partition-group tile
    QUADS = [list(range(q, min(q + QG, BL))) for q in range(0, BL, QG)]
    HALVES = ((0, 4), (4, 3)) if True else None  # pr-rows split 4+3

    with tile.TileContext(nc) as tc, ExitStack() as top:
        pers = top.enter_context(tc.tile_pool(name="pers", bufs=1))
        pst = top.enter_context(tc.tile_pool(name="pst", bufs=1, space="PSUM"))

        lat = [pers.tile([128, NP], f32r, tag=f"lat{m}", name=f"lat{m}") for m in range(MT)]
        latb = [pers.tile([128, NP], bf16, tag=f"latb{m}", name=f"latb{m}") for m in range(MT)]

        # ---------------- encoder ----------------
        with ExitStack() as enc:
            ew = enc.enter_context(tc.tile_pool(name="encw", bufs=1))
            ep = enc.enter_context(tc.tile_pool(name="enc", bufs=2))
            hp = enc.enter_context(tc.tile_pool(name="hp", bufs=2))
            hcp = enc.enter_context(tc.tile_pool(name="hcp", bufs=6))
            ps1 = enc.enter_context(tc.tile_pool(name="ps1", bufs=3, space="PSUM"))
            ps2 = enc.enter_context(tc.tile_pool(name="ps2", bufs=2, space="PSUM"))

            w1_sb = ew.tile([128, 8, 256], bf16)
            for g in range(QG):
                nc.sync.dma_start(
                    out=w1_sb[32 * g:32 * g + 24], in_=w1r[:, :, :]
                )
            b1d = ew.tile([128, 2], f32)
            nc.sync.dma_start(out=b1d, in_=b1p[:, :])
            nc.vector.tensor_scalar_mul(b1d, b1d, 1.0 / 64.0)
            w2a = ew.tile([128, LAT], f32r)
            w2b = ew.tile([128, LAT], f32r)
            nc.sync.dma_start(out=w2a, in_=w2t[0:128, :])
            nc.sync.dma_start(out=w2b, in_=w2t[128:256, :])
            b2d = ew.tile([128, MT], f32)
            nc.sync.dma_start(out=b2d, in_=b2[:, :])
            nc.vector.tensor_scalar_mul(b2d, b2d, 1.0 / 64.0)

            for quad in QUADS:
                for pr0, npr in HALVES:
                    # Brow: partition group 32g holds image quad[g], rows
                    # (c*8+kh); free = (y8, col): deduped 8-row-blocks.
                    rows4 = 4 * npr + 4
                    Brow = ep.tile([128, 20, 256], bf16, tag="Brow")
                    # B32: per-patch expanded+masked view, free (pr,pc,oh,x)
                    B32 = ep.tile([128, npr, 8, 7, 64], bf16, tag="B32")
                    RM32 = ep.tile([128, npr, 7, 8], bf16, tag="RM32")
                    CM32 = ep.tile([128, npr, 7, 64], bf16, tag="CM32")
                    if cfg.sim_safe:
                        nc.gpsimd.memset(Brow, 0.0)
                        nc.gpsimd.memset(RM32, 0.0)
                        nc.gpsimd.memset(CM32, 0.0)
                    for g, img in enumerate(quad):
                        for c in range(3):
                            ioff = img * 196608 + c * 65536 + 4 * pr0 * 256
                            deng = nc.sync if c % 2 == 0 else nc.scalar
                            deng.dma_start(
                                out=Brow[32 * g + 8 * c:32 * g + 8 * c + 8, 0:rows4],
                                in_=_apv(
                                    images[:, :, :, :, :],
                                    [[8192, 8], [256, rows4], [1, 256]],
                                    ioff,
                                ),
                            )
                        moff = (img * 24) * 392 + pr0 * 56
                        nc.scalar.dma_start(
                            out=RM32[32 * g:32 * g + 24],
                            in_=_apv(rowmx[:, :, :], [[392, 24], [1, npr * 56]], moff),
                        )
                        moff = (img * 24) * 3136 + pr0 * 448
                        nc.scalar.dma_start(
                            out=CM32[32 * g:32 * g + 24],
                            in_=_apv(colmx[:, :, :], [[3136, 24], [1, npr * 448]], moff),
                        )
                    # masked = (B+1)*RM*CM; pass 1 also expands rows to
                    # overlapping patches via an overlapping-stride read view
                    # (one 3D op per (pr, oh) - walrus limits stt to 3D).
                    # B32 free order is (pr, oh, pc, x) so pass 2's column
                    # mask broadcasts over oh in ONE 3D op per pr.
                    brf = Brow[:, :, :]
                    rmf = RM32[:, :, :, :]
                    cmf = CM32[:, :, :, :]
                    b32f = B32[:, :, :, :, :]
                    for prl in range(npr):
                        for oh in range(8):
                            nc.vector.scalar_tensor_tensor(
                                out=B32[:, prl, oh, :, :], scalar=1.0,
                                in0=_apv(brf, [brf.ap[0], [32, 7], [1, 64]],
                                         (4 * prl + oh) * 256),
                                in1=_apv(rmf, [rmf.ap[0], [8, 7], [0, 64]],
                                         prl * 56 + oh),
                                op0=AL.add, op1=AL.mult,
                            )
                        b32sl = _apv(b32f, [b32f.ap[0], [448, 8], [1, 448]],
                                     prl * 3584)
                        nc.vector.scalar_tensor_tensor(
                            out=b32sl, in0=b32sl, scalar=0.0,
                            in1=_apv(cmf, [cmf.ap[0], [0, 8], [1, 448]],
                                     prl * 448),
                            op0=AL.add, op1=AL.mult,
                        )
                    pairs = [(0, 1), (2, 3)] if npr == 4 else [(0, 1), (2,)]
                    for g, img in enumerate(quad):
                        bsl = B32[32 * g:32 * g + 24]
                        for pair in pairs:
                            # conv1 for the pr-pair; h pre-scaled by 1/64 so
                            # the conv2 epilogue is a plain bias+relu
                            h_pp = {}
                            for j, prl in enumerate(pair):
                                for cot in range(2):
                                    ps = ps1.tile([128, 448], f32, tag="c1")
                                    for kw in range(8):
                                        rhs = _apv(
                                            bsl,
                                            [bsl.ap[0], [64, 7], [448, 8], [8, 8]],
                                            prl * 3584 + kw,
                                        )
                                        nc.tensor.matmul(
                                            ps,
                                            w1_sb[32 * g:32 * g + 24, kw,
                                                  cot * 128:(cot + 1) * 128],
                                            rhs, start=(kw == 0), stop=(kw == 7),
                                            tile_position=(32 * g, 0),
                                        )
                                    h = hp.tile([128, 448], f32r, tag=f"h{j}{cot}")
                                    nc.scalar.activation(
                                        h, ps, AF.Relu, bias=b1d[:, cot:cot + 1],
                                        scale=1.0 / 64.0,
                                    )
                                    h_pp[(j, cot)] = h
                            base = img * 49 + (pr0 + pair[0]) * 7
                            w = 7 * len(pair)
                            for m in range(MT):
                                ps = ps2.tile([128, 2, 512], f32, tag="c2")
                                for j, prl in enumerate(pair):
                                    _mmr(nc,
                                        ps[:, j, 0:448],
                                        w2a[:, m * 128:(m + 1) * 128], h_pp[(j, 0)],
                                        start=True, stop=False,
                                    )
                                    _mmr(nc,
                                        ps[:, j, 0:448],
                                        w2b[:, m * 128:(m + 1) * 128], h_pp[(j, 1)],
                                        start=False, stop=True,
                                    )
                                psf = ps[:, :, :]
                                psv = _apv(psf, [psf.ap[0], [512, len(pair)], [1, 448]])
                                hc = hcp.tile([128, 2, 7, 64], bf16, tag="hc")
                                hcf = hc[:, :, :, :]
                                hcv = _apv(hcf, [hcf.ap[0], [448, len(pair)], [1, 448]])
                                eng = nc.scalar
                                if eng is nc.scalar:
                                    nc.scalar.activation(
                                        hcv, psv, AF.Relu, bias=b2d[:, m:m + 1],
                                        scale=1.0,
                                    )
                                else:
                                    eng.tensor_scalar(
                                        hcv, psv, b2d[:, m:m + 1], 0.0,
                                        op0=AL.add, op1=AL.max,
                                    )
                                with nc.allow_low_precision(
                                    reason="64-elem pool; bf16 out is plenty"
                                ):
                                    nc.vector.reduce_sum(
                                        out=latb[m][:, base:base + w],
                                    in_=_apv(hcf, [hcf.ap[0], [64, w], [1, 64]]),
                                    axis=AX.X,
                                )

        for m in range(MT):
            nc.vector.tensor_copy(lat[m][:, :], latb[m][:, :])

        # ---------------- latents mean + targets (+ gather) ----------------
        z2t_sb = pers.tile([128, KC, 64], f32r)
        nc.sync.dma_start(
            out=z2t_sb, in_=_apv(z2tt[:, :], [[64, 128], [8192, KC], [1, 64]])
        )
        z2tb_sb = pers.tile([64, 1], f32)
        nc.sync.dma_start(out=z2tb_sb, in_=_apv(z2tb[:], [[1, 64], [0, 1]]))
        for m in range(MT):
            lm = pers.tile([128, BL], f32, tag="lm", name="lm", bufs=2)
            nc.vector.reduce_sum(
                out=lm,
                in_=lat[m][:, :].rearrange("p (b s) -> p b s", s=49),
                axis=AX.X,
            )
            nc.vector.tensor_scalar_mul(lm, lm, 1.0 / 49.0)
            nc.sync.dma_start(out=out_lm[m * 128:(m + 1) * 128, :], in_=lm)

        psT = pst.tile([64, NP], f32, tag="pt")
        for kc in range(KC):
            _mmr(nc,
                psT, z2t_sb[:, kc, :], lat[kc][:, :],
                start=(kc == 0), stop=(kc == KC - 1),
            )
        T_sb = pers.tile([64, NP], f32r)
        nc.scalar.activation(T_sb, psT, AF.Identity, bias=z2tb_sb[:, 0:1], scale=1.0)
        nc.sync.dma_start(out=tg_in[:, :], in_=T_sb)
        if cfg.use_collective:
            nc.gpsimd.collective_compute(
                "AllGather",
                AL.bypass,
                replica_groups=[list(range(cfg.ncores))],
                ins=[tg_in[:, :]],
                outs=[tg_out[:, :, :]],
            )
        else:
            nc.gpsimd.dma_start(out=tg_out[0], in_=tg_in[:, :])

        # ---------------- pixelcnn ----------------
        x = list(lat)
        with ExitStack() as pcs:
            pw = pcs.enter_context(tc.tile_pool(name="pcw", bufs=2))
            yp = pcs.enter_context(tc.tile_pool(name="yp", bufs=2))
            ps3 = pcs.enter_context(tc.tile_pool(name="ps3", bufs=3, space="PSUM"))

            pb1_sb = pers.tile([128, NBLK, 2], f32)
            nc.sync.dma_start(
                out=pb1_sb, in_=pcb1[:, :, :]
            )
            pb2_sb = pers.tile([128, NBLK, 2], f32)
            nc.sync.dma_start(
                out=pb2_sb, in_=pcb2[:, :, :]
            )
            pb3_sb = pers.tile([128, NBLK, 2], f32)
            nc.sync.dma_start(
                out=pb3_sb, in_=pcb3[:, :, :]
            )
            pb4_sb = pers.tile([128, NBLK, MT], f32)
            nc.sync.dma_start(
                out=pb4_sb, in_=pcb4[:, :, :]
            )

            for k in range(NBLK):
                pc1_sb = pw.tile([128, KC, 256], f32r, tag="pc1")
                nc.sync.dma_start(
                    out=pc1_sb,
                    in_=_apv(
                        pc1t[:, :, :], [[256, 128], [32768, KC], [1, 256]],
                        k * LAT * 256,
                    ),
                )
                pc2_sb = pw.tile([128, 3, 2, 256], f32r, tag="pc2")
                for dx in range(3):
                    nc.sync.dma_start(
                        out=pc2_sb[:, dx],
                        in_=_apv(
                            pc2t[:, :, :, :],
                            [[256, 128], [32768, 2], [1, 256]],
                            (k * 3 + dx) * 65536,
                        ),
                    )
                pc3_sb = pw.tile([128, 2, 2, 256], f32r, tag="pc3")
                for dy in range(2):
                    nc.sync.dma_start(
                        out=pc3_sb[:, dy],
                        in_=_apv(
                            pc3t[:, :, :, :],
                            [[256, 128], [32768, 2], [1, 256]],
                            (k * 2 + dy) * 65536,
                        ),
                    )
                pc4_sb = pw.tile([128, 2, LAT], f32r, tag="pc4")
                nc.sync.dma_start(
                    out=pc4_sb,
                    in_=_apv(
                        pc4t[:, :, :], [[LAT, 128], [128 * LAT, 2], [1, LAT]],
                        k * 256 * LAT,
                    ),
                )

                # y1 = relu(1x1 conv LAT->256), written into col-padded buffer
                y1p = []
                for m2 in range(2):
                    ps = ps3.tile([128, NP], f32, tag="py")
                    for kc in range(KC):
                        _mmr(nc,
                            ps, pc1_sb[:, kc, m2 * 128:(m2 + 1) * 128], x[kc][:, :],
                            start=(kc == 0), stop=(kc == KC - 1),
                        )
                    t = yp.tile([128, BL, 7, 9], f32r, tag=f"y1p{m2}")
                    nc.vector.memset(t[:, :, :, :].bitcast(f32), 0.0)
                    nc.scalar.activation(
                        t[:, :, :, 1:8], ps, AF.Relu,
                        bias=pb1_sb[:, k, m2:m2 + 1], scale=1.0,
                    )
                    y1p.append(t)
                # y2 = relu(1x3 conv along columns), row-padded buffer
                y2p = []
                for m2 in range(2):
                    ps = ps3.tile([128, NP], f32, tag="py")
                    first = True
                    for dx in range(3):
                        for kc in range(2):
                            nc.tensor.matmul(
                                ps,
                                pc2_sb[:, dx, kc, m2 * 128:(m2 + 1) * 128].bitcast(f32),
                                y1p[kc][:, :, :, dx:dx + 7].bitcast(f32),
                                start=first, stop=(dx == 2 and kc == 1),
                            )
                            first = False
                    t = yp.tile([128, BL, 8, 7], f32r, tag=f"y2p{m2}")
                    nc.vector.memset(t[:, :, :, :].bitcast(f32), 0.0)
                    nc.scalar.activation(
                        t[:, :, 1:8, :], ps, AF.Relu,
                        bias=pb2_sb[:, k, m2:m2 + 1], scale=1.0,
                    )
                    y2p.append(t)
                # y3 = relu(2x1 conv along rows, top pad)
                y3 = []
                for m2 in range(2):
                    ps = ps3.tile([128, NP], f32, tag="py")
                    first = True
                    for dy in range(2):
                        for kc in range(2):
                            nc.tensor.matmul(
                                ps,
                                pc3_sb[:, dy, kc, m2 * 128:(m2 + 1) * 128].bitcast(f32),
                                y2p[kc][:, :, dy:dy + 7, :].bitcast(f32),
                                start=first, stop=(dy == 1 and kc == 1),
                            )
                            first = False
                    t = yp.tile([128, NP], f32r, tag=f"y3{m2}")
                    nc.scalar.activation(
                        t, ps, AF.Relu, bias=pb3_sb[:, k, m2:m2 + 1], scale=1.0
                    )
                    y3.append(t)
                # y4 = 1x1 conv 256->LAT; x = relu(y4 + b4 + x)
                for m in range(MT):
                    ps = ps3.tile([128, NP], f32, tag="py4")
                    for kc in range(2):
                        _mmr(nc,
                            ps, pc4_sb[:, kc, m * 128:(m + 1) * 128], y3[kc][:, :],
                            start=(kc == 0), stop=(kc == 1),
                        )
                    t = yp.tile([128, NP], f32, tag="resid")
                    nc.vector.scalar_tensor_tensor(
                        out=t, in0=ps, scalar=pb4_sb[:, k, m:m + 1], in1=x[m][:, :],
                        op0=AL.add, op1=AL.add,
                    )
                    nc.scalar.activation(x[m][:, :], t, AF.Relu)

        # ---------------- preds + logits + loss pieces ----------------
        with ExitStack() as lgs:
            lp = lgs.enter_context(tc.tile_pool(name="lp", bufs=2))
            lp1 = lgs.enter_context(tc.tile_pool(name="lp1", bufs=1))
            ps4 = lgs.enter_context(tc.tile_pool(name="ps4", bufs=2, space="PSUM"))

            c2p_sb = lp1.tile([128, 3, KC, 64], f32r)
            for si in range(3):
                nc.sync.dma_start(
                    out=c2p_sb[:, si],
                    in_=_apv(c2pt[:, :, :], [[64, 128], [8192, KC], [1, 64]],
                             si * LAT * 64),
                )
            c2pb_sb = lp1.tile([64, 3], f32)
            nc.sync.dma_start(out=c2pb_sb, in_=c2pb[:, :])

            preds_cat = lp1.tile([64, ROWS], f32r)
            for si in range(3):
                psP = ps4.tile([64, NP], f32, tag="pp")
                for kc in range(KC):
                    _mmr(nc,
                        psP, c2p_sb[:, si, kc, :], x[kc][:, :],
                        start=(kc == 0), stop=(kc == KC - 1),
                    )
                P_sb = lp.tile([64, BL, 7, 7], f32, tag="P")
                nc.scalar.activation(
                    P_sb, psP, AF.Identity, bias=c2pb_sb[:, si:si + 1], scale=1.0
                )
                n_i = BL * HI[si] * 7
                nc.vector.tensor_copy(
                    preds_cat[:, OFFS[si]:OFFS[si] + n_i],
                    P_sb[:, :, 0:HI[si], :],
                )
            # label logits: preds . target[label]; labels are same-image patches
            prod = lp1.tile([64, ROWS], f32r)
            Tr = T_sb[:, :].rearrange("t (b r c) -> t b r c", r=7, c=7)
            for si in range(3):
                n_i = BL * HI[si] * 7
                nc.vector.tensor_mul(
                    prod[:, OFFS[si]:OFFS[si] + n_i],
                    preds_cat[:, OFFS[si]:OFFS[si] + n_i],
                    Tr[:, :, si + 3:si + 3 + HI[si], :],
                )
            ones_sb = lp1.tile([64, 1], f32r)
            nc.vector.memset(ones_sb[:, :].bitcast(f32), 1.0)
            psL = ps4.tile([1, ROWS], f32, tag="pl")
            _mmr(nc, psL, ones_sb, prod, start=True, stop=True)
            ll_sb = lp1.tile([1, ROWS], f32)
            nc.scalar.copy(ll_sb, psL)
            nc.sync.dma_start(out=out_ll[:], in_=ll_sb)

            # logits against all gathered targets, streamed row-block-wise
            Tfull = lp1.tile([64, cfg.ncores, NP], f32r)
            nc.sync.dma_start(
                out=Tfull,
                in_=tg_out[:, :, :].rearrange("c t p -> t c p"),
            )
            tf = Tfull[:, :, :]
            tflat = bass.AP(tensor=tf.tensor, offset=tf.offset, ap=[tf.ap[0], [1, GC]])
            n_mb = 4 if ROWS % 4 == 0 and ROWS // 4 <= 128 else 1
            MB = ROWS // n_mb
            csz = 448
            n_ch = (GC + csz - 1) // csz
            for m4 in range(n_mb):
                lg = lp.tile([MB, GC], f32, tag="lg")
                for nch in range(n_ch):
                    w = min(csz, GC - nch * csz)
                    ps = ps4.tile([MB, csz], f32, tag="plg")
                    _mmr(nc,
                        ps[:, 0:w],
                        preds_cat[:, m4 * MB:(m4 + 1) * MB],
                        _apv(tflat, [tflat.ap[0], [1, w]], nch * csz),
                        start=True, stop=True,
                    )
                    nc.scalar.copy(lg[:, nch * csz:nch * csz + w], ps[:, 0:w])
                mx = lp.tile([MB, 1], f32, tag="mx")
                nc.vector.reduce_max(out=mx, in_=lg, axis=AX.X)
                nmx = lp.tile([MB, 1], f32, tag="nmx")
                nc.vector.tensor_scalar_mul(nmx, mx, -1.0)
                ex = lp.tile([MB, GC], f32, tag="ex")
                nc.scalar.activation(ex, lg, AF.Exp, bias=nmx, scale=1.0)
                zz = lp.tile([MB, 1], f32, tag="zz")
                nc.vector.reduce_sum(out=zz, in_=ex, axis=AX.X)
                lnz = lp.tile([MB, 1], f32, tag="lnz")
                nc.scalar.activation(lnz, zz, AF.Ln)
                mlz = lp.tile([MB, 1], f32, tag="mlz")
                nc.vector.tensor_add(mlz, mx, lnz)
                nc.sync.dma_start(out=out_mlz[m4 * MB:(m4 + 1) * MB], in_=mlz)

    return nc


# ---------------------------------------------------------------------------


def host_prep(inputs, cfg: Cfg):
    """Per-core input maps. Only sharding, layout transforms of weights, and
    index-arithmetic mask vectors happen here — all FLOPs stay on device."""
    LAT, NBLK = cfg.latent, cfg.nblk
    EMB_SCALE = 0.1
    images = np.ascontiguousarray(inputs["images"], dtype=np.float32)
    rnd = np.asarray(inputs["rnd"]).astype(np.int32)
    B = images.shape[0]

    r0 = (rnd // 4).astype(np.int64)
    c0 = (rnd % 4).astype(np.int64)
    idx = np.arange(64)
    rowm_all = ((idx[None, :] >= r0[:, None]) & (idx[None, :] < r0[:, None] + 60)).astype(np.float32)
    colm_all = ((idx[None, :] >= c0[:, None]) & (idx[None, :] < c0[:, None] + 60)).astype(np.float32)
    # device layouts: rowmx[b, c*8+kh, (pr,pc,oh)] = rowm[p, 8*oh+kh]
    #                 colmx[b, c*8+kh, (pr,pc,x)]  = colm[p, x]
    rm_p = rowm_all.reshape(B, 49, 8, 8)                       # (b, p, oh, kh)
    rowmx_all = np.broadcast_to(
        rm_p.transpose(0, 3, 1, 2)[:, None, :, :, :], (B, 3, 8, 49, 8)
    ).reshape(B, 24, 392).astype(np.float32)
    cm_p = colm_all.reshape(B, 49, 64)
    colmx_all = np.broadcast_to(
        cm_p[:, None, None, :, :], (B, 3, 8, 49, 64)
    ).reshape(B, 24, 3136).astype(np.float32)

    w1 = np.asarray(inputs["enc_w1"], dtype=np.float32)          # (256,3,8,8)
    w1r = np.ascontiguousarray(w1.transpose(1, 2, 3, 0).reshape(24, 8, 256))
    b1p = (np.asarray(inputs["enc_b1"], np.float32) - w1.sum(axis=(1, 2, 3)))
    w2t = np.ascontiguousarray(np.asarray(inputs["enc_w2"], np.float32)[:, :, 0, 0].T)  # (256,LAT)
    b2 = np.asarray(inputs["enc_b2"], np.float32)
    pc1t = np.ascontiguousarray(np.asarray(inputs["pc_w1"], np.float32)[:, :, :, 0, 0].transpose(0, 2, 1))  # (5,LAT,256)
    pc2t = np.ascontiguousarray(np.asarray(inputs["pc_w2"], np.float32)[:, :, :, 0, :].transpose(0, 3, 2, 1))  # (5,3,256in,256out)
    pc3t = np.ascontiguousarray(np.asarray(inputs["pc_w3"], np.float32)[:, :, :, :, 0].transpose(0, 3, 2, 1))  # (5,2,256in,256out)
    pc4t = np.ascontiguousarray(np.asarray(inputs["pc_w4"], np.float32)[:, :, :, 0, 0].transpose(0, 2, 1))  # (5,256,LAT)
    z2tt = np.ascontiguousarray(np.asarray(inputs["z2t_w"], np.float32)[:, :, 0, 0].T)  # (LAT,64)
    c2pt = np.ascontiguousarray(np.asarray(inputs["c2p_w"], np.float32)[:, :, :, 0, 0].transpose(0, 2, 1)) * EMB_SCALE
    c2pb = np.asarray(inputs["c2p_b"], np.float32) * EMB_SCALE

    MT = cfg.mt
    mkb = lambda v, n: np.ascontiguousarray(
        np.asarray(v, np.float32).reshape(NBLK, n, 128).transpose(2, 0, 1)
    )
    import ml_dtypes
    bfl = ml_dtypes.bfloat16
    shared = {
        "w1r": w1r.astype(bfl),
        "b1p": np.ascontiguousarray(b1p.reshape(2, 128).T),
        "w2t": w2t,
        "b2": np.ascontiguousarray(b2.reshape(MT, 128).T),
        "pc1t": pc1t, "pcb1": mkb(inputs["pc_b1"], 2),
        "pc2t": pc2t, "pcb2": mkb(inputs["pc_b2"], 2),
        "pc3t": pc3t, "pcb3": mkb(inputs["pc_b3"], 2),
        "pc4t": pc4t, "pcb4": mkb(inputs["pc_b4"], MT),
        "z2tt": z2tt, "z2tb": np.asarray(inputs["z2t_b"], np.float32),
        "c2pt": c2pt, "c2pb": np.ascontiguousarray(c2pb.T),
    }
    in_maps = []
    for c in range(cfg.ncores):
        bsl = slice(c * cfg.bl, (c + 1) * cfg.bl)
        m = dict(shared)
        m["images"] = np.ascontiguousarray(
            images[bsl].reshape(cfg.bl, 3, 32, 8, 256).transpose(0, 1, 3, 2, 4)
        ).astype(bfl)
        m["rowmx"] = np.ascontiguousarray(rowmx_all[bsl]).astype(bfl)
        m["colmx"] = np.ascontiguousarray(colmx_all[bsl]).astype(bfl)
        in_maps.append(m)
    return in_maps


def host_epilogue(results, cfg: Cfg):
    B, BL = cfg.ncores * cfg.bl, cfg.bl
    HI = (4, 3, 2)
    OFFS = (0, BL * 28, BL * 49, BL * 63)
    loss = np.float64(0.0)
    for si in range(3):
        tot = B * HI[si] * 7
        s = np.float64(0.0)
        for r in results:
            sl = slice(OFFS[si], OFFS[si + 1])
            s += (r["out_mlz"][sl].astype(np.float64) - r["out_ll"][sl].astype(np.float64)).sum()
        loss += s / tot
    latmean = np.concatenate([r["out_lm"].T for r in results], axis=0)  # (B, LAT)
    return np.float32(loss), latmean.astype(np.float32)


_CACHE = {}


def kernel(**inputs):
    cfg = Cfg()
    if "nc" not in _CACHE:
        _CACHE["nc"] = build_nc(cfg)
    nc = _CACHE["nc"]
    in_maps = host_prep(inputs, cfg)
    from concourse.bass_utils import run_bass_kernel_spmd

    if not _CACHE.get("split_done"):
        _split_multiwait(nc)
        _CACHE["split_done"] = True
    res = run_bass_kernel_spmd(nc, in_maps, list(range(cfg.ncores)))
    return host_epilogue(res.results, cfg)
